# revision 7
# baseline (speedup 1.0000x reference)
"""Trainium2 Bass kernel for nn_CustomCNNLayer_84559316124470.

The reference computes, per batch b:
    win[b,c,s,m]   = xp[b,c,s+m]                    (xp = x padded with K-1 zeros)
    xw[b,c,s,m,l]  = win[b,c,s,m] * stft_w[l,m]
    xr             = xw.reshape(b, c*K*NK, s)       (raw row-major reshape)
    out            = relu(conv_w @ xr + bias)       (1x1 conv over channels)

Because K*NK == S/2 == 2048, the raw reshape maps
    xr[b, c*2048 + q, p*2048 + m*32 + l] = xp[b, c, 2q+p+m] * stft_w[l, m]
(with s = 2q+p). Hence, with h[b,o,r] = sum_{c,q} conv_w[o, c*2048+q] * xp[b,c,2q+r]
(r in [0, 65)):
    out[b, o, p*2048 + m*32 + l] = relu(stft_w[l,m] * h[b,o,p+m] + bias[o])

So the dense 8.6 GMAC/batch matmul collapses to a (512x4096)@(4096x65)
strided correlation (tensor engine) plus a per-element broadcast expansion
(vector engine) and bias+ReLU (scalar/vector engines).

Sharding: output channels o split across the 8 cores (64 rows each);
window matrices replicated. No collectives.

Precision: mm1 runs on the PE in bf16. With PASSES=3 the fp32 operands are
split hi/lo into bf16 pairs and three accumulating matmuls recover ~fp32
accuracy (error ~1e-5 rel. vs ~1e-3 for PASSES=1); fp32 PE matmuls run in
multi-pass LOW_HIGH mode and are not competitive.
"""

import numpy as np
import ml_dtypes

import concourse.bass as bass
import concourse.tile as tile
from concourse import bacc, mybir
from concourse.bass_utils import run_bass_kernel_spmd

B, C, S = 4, 2, 4096
K, NK, OUT = 64, 32, 512
Q = K * NK            # 2048 == S // 2
R = K + 1             # 65 shift taps
NCORES = 8
OSH = OUT // NCORES   # 64 output channels per core
KT = 32               # contraction tiles of 128 over c*Q = 4096
W260 = B * R          # per-kt rhs free dim: [z=0 | z=1] x [pair 0 | pair 1] x r
PASSES = 3            # 1 = plain bf16, 3 = hi/lo split (near-fp32)
NCH = 4               # DMA chunks over kt
KTC = KT // NCH
NPASS_COLS = (2 * OSH + 2 * W260) if PASSES == 3 else (OSH + W260)
CHUNK_W = KTC * NPASS_COLS  # columns per chunk in the merged bf16 buffer
F32 = mybir.dt.float32
BF16 = mybir.dt.bfloat16

_PROGRAM = None
_LAST_RESULTS = None


def _kernel_body(tc, out, ins):
    nc = tc.nc
    from contextlib import ExitStack

    with ExitStack() as ctx:
        const = ctx.enter_context(tc.tile_pool(name="const", bufs=1))
        psum_h = ctx.enter_context(tc.tile_pool(name="psum_h", bufs=1, space="PSUM"))
        tmp_p = ctx.enter_context(tc.tile_pool(name="tmp_p", bufs=2))
        sbuf_o = ctx.enter_context(tc.tile_pool(name="sbuf_o", bufs=3))

        # chunk layout (bf16): [ch (KTC*64) | cl | xh (KTC*260) | xl]
        co, xo = 0, (2 if PASSES == 3 else 1) * KTC * OSH
        b_sb = const.tile([128, 1], F32, tag="b_sb")
        nc.scalar.dma_start(b_sb[:], ins["bias2"])
        chunks = []
        for chk in range(NCH):
            w_t = const.tile([128, CHUNK_W], BF16, tag=f"w{chk}")
            nc.sync.dma_start(w_t[:], ins["wbuf"][:, bass.ts(chk, CHUNK_W)])
            chunks.append(w_t)
            if chk == 1:
                T_sb = const.tile([128, Q], F32, tag="T_sb")
                nc.scalar.dma_start(T_sb[:], ins["trow"].to_broadcast((128, Q)))

        # mm1: h[o', z*130 + pr*65 + r] = sum_g conv_w[o_shard+o', g]*X_{2pr+z}[g,r]
        pass_offs = [(co, xo)] if PASSES == 1 else [
            (co, xo),                      # ch @ xh
            (co, xo + KTC * W260),         # ch @ xl
            (co + KTC * OSH, xo),          # cl @ xh
        ]
        h_ps = psum_h.tile([OSH, W260], F32, tag="h_ps")
        n_mm = NCH * KTC * len(pass_offs)
        i_mm = 0
        for chk in range(NCH):
            for kt in range(KTC):
                for c_off, x_off in pass_offs:
                    nc.tensor.matmul(
                        h_ps[:],
                        chunks[chk][:, c_off + kt * OSH : c_off + (kt + 1) * OSH],
                        chunks[chk][:, x_off + kt * W260 : x_off + (kt + 1) * W260],
                        start=(i_mm == 0),
                        stop=(i_mm == n_mm - 1),
                    )
                    i_mm += 1

        # redistribute h (64, [z|pr|r]) -> h2 (z*64+o', pr*65+r) via sb2sb DMA
        h_sb = const.tile([OSH, W260], F32, tag="h_sb")
        nc.vector.tensor_copy(h_sb[:], h_ps[:])
        h2_sb = const.tile([128, 2 * R], F32, tag="h2_sb")
        for z, eng in ((0, nc.sync), (1, nc.scalar)):
            eng.dma_start(
                h2_sb[z * OSH : (z + 1) * OSH, :],
                h_sb[:, z * 2 * R : (z + 1) * 2 * R],
            )

        # expansion: out[z*64+o', u] = relu(h2[z*64+o', pr*65+p+u//32]*T[u] + bias)
        # multiply groups 0-2 on DVE, 3 on GpSimd; bias+relu 0-2 on ACT, 3 on DVE
        for g, (pr, p) in enumerate([(0, 0), (0, 1), (1, 0), (1, 1)]):
            off = pr * R + p
            h_exp = h2_sb[:, off : off + K].unsqueeze(2).to_broadcast(
                (128, K, NK)
            )
            tmp = tmp_p.tile([128, Q], F32, tag="tmp")
            mul_eng = nc.gpsimd if g == 3 else nc.vector
            mul_eng.tensor_tensor(
                tmp.rearrange("a (m l) -> a m l", l=NK),
                h_exp,
                T_sb.rearrange("a (m l) -> a m l", l=NK),
                mybir.AluOpType.mult,
            )
            o_sb = sbuf_o.tile([128, Q], F32, tag="o_sb")
            if g < 3:
                nc.scalar.activation(
                    o_sb[:], tmp[:], mybir.ActivationFunctionType.Relu,
                    bias=b_sb[:],
                )
            else:
                nc.vector.tensor_scalar(
                    o_sb[:], tmp[:], b_sb[:], 0.0,
                    mybir.AluOpType.add, mybir.AluOpType.max,
                )
            for half, eng in ((0, nc.sync), (1, nc.scalar)):
                eng.dma_start(
                    out[
                        pr * 128 : (pr + 1) * 128,
                        p * Q + half * (Q // 2) : p * Q + (half + 1) * (Q // 2),
                    ],
                    o_sb[:, bass.ts(half, Q // 2)],
                )


def _build_program():
    nc = bacc.Bacc(
        "TRN2", target_bir_lowering=False, debug=False, num_devices=NCORES
    )
    ins = {}
    ins["wbuf"] = nc.dram_tensor(
        "wbuf", [128, NCH * CHUNK_W], BF16, kind="ExternalInput"
    ).ap()
    ins["trow"] = nc.dram_tensor("trow", [1, Q], F32, kind="ExternalInput").ap()
    ins["bias2"] = nc.dram_tensor("bias2", [128, 1], F32, kind="ExternalInput").ap()
    out = nc.dram_tensor("out", [2 * 128, S], F32, kind="ExternalOutput").ap()

    with tile.TileContext(nc) as tc:
        _kernel_body(tc, out, ins)
    nc.compile()
    return nc


def _split_bf16(a):
    hi = a.astype(ml_dtypes.bfloat16)
    lo = (a - hi.astype(np.float32)).astype(ml_dtypes.bfloat16)
    return hi, lo


def _host_prepare(x, stft_w, conv_w, conv_b):
    """Build per-core input maps."""
    x = np.ascontiguousarray(x, dtype=np.float32)
    xp = np.zeros((B, C, 2 * Q + K), dtype=np.float32)  # padded to 4160
    xp[:, :, :S] = x
    sb_, sc_, ss_ = xp.strides
    win = np.lib.stride_tricks.as_strided(
        xp, shape=(B, C, Q, R), strides=(sb_, sc_, 2 * ss_, ss_)
    )
    Xf = win.reshape(B, C * Q, R)                      # (4, 4096, 65), b=2*pr+z
    # layout [p, kt, z, pr, r]: batch order (z,pr) -> b = [0, 2, 1, 3]
    X5 = np.ascontiguousarray(
        Xf[[0, 2, 1, 3]].reshape(2, 2, KT, 128, R).transpose(3, 2, 0, 1, 4)
    ).reshape(128, KT, W260)
    xh, xl = _split_bf16(X5)

    trow = np.ascontiguousarray(stft_w.T, dtype=np.float32).reshape(1, Q)

    in_maps = []
    for i in range(NCORES):
        cw_sh = conv_w[i * OSH : (i + 1) * OSH, :]     # (64, 4096)
        cwt = np.ascontiguousarray(
            cw_sh.reshape(OSH, KT, 128).transpose(2, 1, 0)  # (128, 32, 64)
        )
        ch, cl = _split_bf16(cwt)
        # merged chunk buffer: per chunk [ch | cl | xh | xl], kt-major inside
        parts = [ch, cl, xh, xl] if PASSES == 3 else [ch, xh]
        wbuf = np.empty((128, NCH, NPASS_COLS * KTC), dtype=ml_dtypes.bfloat16)
        for chk in range(NCH):
            sl = slice(chk * KTC, (chk + 1) * KTC)
            wbuf[:, chk, :] = np.concatenate(
                [p_[:, sl].reshape(128, -1) for p_ in parts], axis=1
            )
        bias2 = np.ascontiguousarray(
            np.tile(conv_b[i * OSH : (i + 1) * OSH], 2).reshape(128, 1),
            dtype=np.float32,
        )
        in_maps.append(
            {
                "wbuf": wbuf.reshape(128, NCH * CHUNK_W),
                "trow": trow,
                "bias2": bias2,
            }
        )
    return in_maps


def kernel(x, stft_w, conv_w, conv_b):
    global _PROGRAM, _LAST_RESULTS
    if _PROGRAM is None:
        _PROGRAM = _build_program()
    in_maps = _host_prepare(
        np.asarray(x), np.asarray(stft_w, dtype=np.float32),
        np.asarray(conv_w, dtype=np.float32), np.asarray(conv_b, dtype=np.float32),
    )
    res = run_bass_kernel_spmd(_PROGRAM, in_maps, list(range(NCORES)))
    _LAST_RESULTS = res
    # per-core out: (256, 4096); rows pr*128 + z*64 + o' -> (b=2*pr+z, o=i*64+o')
    full = np.empty((B, OUT, S), dtype=np.float32)
    for i in range(NCORES):
        full[:, i * OSH : (i + 1) * OSH, :] = res.results[i]["out"].reshape(
            B, OSH, S
        )
    return full


if __name__ == "__main__":
    rng = np.random.default_rng(0)
    out = kernel(
        rng.standard_normal((B, C, S), dtype=np.float32),
        rng.standard_normal((NK, K), dtype=np.float32),
        (rng.standard_normal((OUT, C * K * NK)) * 0.02).astype(np.float32),
        (rng.standard_normal((OUT,)) * 0.02).astype(np.float32),
    )
    print(out.shape, out.dtype, float(np.abs(out).max()))


# revision 11
# speedup vs baseline: 1.0142x; 1.0142x over previous
"""Trainium2 Bass kernel for nn_CustomCNNLayer_84559316124470.

The reference computes, per batch b:
    win[b,c,s,m]   = xp[b,c,s+m]                    (xp = x padded with K-1 zeros)
    xw[b,c,s,m,l]  = win[b,c,s,m] * stft_w[l,m]
    xr             = xw.reshape(b, c*K*NK, s)       (raw row-major reshape)
    out            = relu(conv_w @ xr + bias)       (1x1 conv over channels)

Because K*NK == S/2 == 2048, the raw reshape maps
    xr[b, c*2048 + q, p*2048 + m*32 + l] = xp[b, c, 2q+p+m] * stft_w[l, m]
(with s = 2q+p). Hence, with h[b,o,r] = sum_{c,q} conv_w[o, c*2048+q] * xp[b,c,2q+r]
(r in [0, 65)):
    out[b, o, p*2048 + m*32 + l] = relu(stft_w[l,m] * h[b,o,p+m] + bias[o])

So the dense 8.6 GMAC/batch matmul collapses to a (512x4096)@(4096x65)
strided correlation (tensor engine) plus a per-element broadcast expansion
(vector/gpsimd engines) and bias+ReLU (scalar/vector engines).

Sharding: output channels o split across the 8 cores (64 rows each);
window matrices replicated. No collectives.

Precision: mm1 runs on the PE in bf16. With PASSES=3 the fp32 operands are
split hi/lo into bf16 pairs and three accumulating matmuls recover ~fp32
accuracy (error ~1e-5 rel.); fp32 PE matmuls run in multi-pass LOW_HIGH
mode and are not competitive.

Raw (non-Tile) implementation: hand-placed semaphores keep the setup and
teardown overhead minimal (Tile's vector-clock epilogue resets every
semaphore individually, ~10us).
"""

import numpy as np
import ml_dtypes

import concourse.bass as bass
from concourse import bacc, mybir
from concourse.bass_utils import run_bass_kernel_spmd

B, C, S = 4, 2, 4096
K, NK, OUT = 64, 32, 512
Q = K * NK            # 2048 == S // 2
R = K + 1             # 65 shift taps
NCORES = 8
OSH = OUT // NCORES   # 64 output channels per core
KT = 32               # contraction tiles of 128 over c*Q = 4096
W260 = B * R          # per-kt rhs free dim: [z=0 | z=1] x [pair 0 | pair 1] x r
PASSES = 3            # 1 = plain bf16, 3 = hi/lo split (near-fp32)
NCH = 4               # DMA chunks over kt
KTC = KT // NCH
NPASS_COLS = (2 * OSH + 2 * W260) if PASSES == 3 else (OSH + W260)
CHUNK_W = KTC * NPASS_COLS
F32 = mybir.dt.float32
BF16 = mybir.dt.bfloat16
GROUPS = [(0, 0), (0, 1), (1, 0), (1, 1)]  # (pr, p)

_PROGRAM = None
_LAST_RESULTS = None


def _build_program():
    nc = bacc.Bacc("TRN2", target_bir_lowering=False, debug=False)
    wbuf = nc.dram_tensor("wbuf", [128, NCH * CHUNK_W], BF16, kind="ExternalInput")
    trow = nc.dram_tensor("trow", [1, Q], F32, kind="ExternalInput")
    bias2 = nc.dram_tensor("bias2", [128, 1], F32, kind="ExternalInput")
    out = nc.dram_tensor("out", [2 * 128, S], F32, kind="ExternalOutput")

    co, xo = 0, (2 if PASSES == 3 else 1) * KTC * OSH
    pass_offs = [(co, xo)] if PASSES == 1 else [
        (co, xo),                      # ch @ xh
        (co, xo + KTC * W260),         # ch @ xl
        (co + KTC * OSH, xo),          # cl @ xh
    ]

    from contextlib import ExitStack

    with ExitStack() as ctx:
        e = ctx.enter_context
        wt = [e(nc.sbuf_tensor(f"wt{c}", [128, CHUNK_W], BF16)) for c in range(NCH)]
        T_sb = e(nc.sbuf_tensor("T_sb", [128, Q], F32))
        b_sb = e(nc.sbuf_tensor("b_sb", [128, 1], F32))
        h_sb = e(nc.sbuf_tensor("h_sb", [OSH, W260], F32))
        h2_sb = e(nc.sbuf_tensor("h2_sb", [128, 2 * R], F32))
        tmp = [e(nc.sbuf_tensor(f"tmp{g}", [128, Q], F32)) for g in range(4)]
        o_sb = [e(nc.sbuf_tensor(f"o{g}", [128, Q], F32)) for g in range(4)]
        h_ps = e(nc.psum_tensor("h_ps", [OSH, W260], F32))

        sin = [e(nc.semaphore(f"sin{c}")) for c in range(NCH)]  # chunk DMAs
        sTa = e(nc.semaphore("sTa"))    # T broadcast DMA
        sTb = e(nc.semaphore("sTb"))    # bias DMA
        spe = e(nc.semaphore("spe"))    # mm1 done
        scp = e(nc.semaphore("scp"))    # h psum->sbuf copy done
        sh2 = e(nc.semaphore("sh2"))    # h2 redistribution DMAs
        stt = e(nc.semaphore("stt"))    # DVE multiplies done (groups 0-2)
        sgp = e(nc.semaphore("sgp"))    # GpSimd multiply done (group 3)
        sact = e(nc.semaphore("sact"))  # ACT relu done (groups 0-2)
        sdv = e(nc.semaphore("sdv"))    # DVE relu done (group 3)
        sout = e(nc.semaphore("sout"))  # out DMAs
        sems = sin + [sTa, sTb, spe, scp, sh2, stt, sgp, sact, sdv, sout]

        def exp_aps(g):
            pr, p = GROUPS[g]
            off = pr * R + p
            h_exp = (
                h2_sb[:, off : off + K].unsqueeze(2).to_broadcast((128, K, NK))
            )
            return (
                tmp[g][:, :].rearrange("a (m l) -> a m l", l=NK),
                h_exp,
                T_sb[:, :].rearrange("a (m l) -> a m l", l=NK),
            )

        with nc.Block() as block:

            @block.sync
            def _(sync):
                for c in range(NCH):
                    sync.dma_start(
                        wt[c][:, :], wbuf[:, c * CHUNK_W : (c + 1) * CHUNK_W]
                    ).then_inc(sin[c], 16)
                sync.wait_ge(scp, 1)
                for z in range(2):
                    sync.dma_start(
                        h2_sb[z * OSH : (z + 1) * OSH, :],
                        h_sb[:, z * 2 * R : (z + 1) * 2 * R],
                    ).then_inc(sh2, 16)
                for g in range(4):
                    pr, p = GROUPS[g]
                    if g < 3:
                        sync.wait_ge(sact, g + 1)
                    else:
                        sync.wait_ge(sdv, 1)
                    sync.dma_start(
                        out[pr * 128 : (pr + 1) * 128, p * Q : (p + 1) * Q],
                        o_sb[g][:, :],
                    ).then_inc(sout, 16)

            @block.scalar
            def _(scalar):
                scalar.dma_start(
                    T_sb[:, :], trow[:, :].to_broadcast((128, Q))
                ).then_inc(sTa, 16)
                scalar.dma_start(b_sb[:, :], bias2[:, :]).then_inc(sTb, 16)
                scalar.wait_ge(sTb, 16)
                for g in range(3):
                    scalar.wait_ge(stt, g + 1)
                    scalar.activation(
                        o_sb[g][:, :], tmp[g][:, :],
                        mybir.ActivationFunctionType.Relu, bias=b_sb[:, :],
                    ).then_inc(sact, 1)

            @block.tensor
            def _(tensor):
                n_mm = NCH * KTC * len(pass_offs)
                i_mm = 0
                for chk in range(NCH):
                    tensor.wait_ge(sin[chk], 16)
                    for kt in range(KTC):
                        for c_off, x_off in pass_offs:
                            mm = tensor.matmul(
                                h_ps[:, :],
                                wt[chk][
                                    :, c_off + kt * OSH : c_off + (kt + 1) * OSH
                                ],
                                wt[chk][
                                    :, x_off + kt * W260 : x_off + (kt + 1) * W260
                                ],
                                start=(i_mm == 0),
                                stop=(i_mm == n_mm - 1),
                            )
                            i_mm += 1
                mm.then_inc(spe, 1)

            @block.vector
            def _(vector):
                vector.wait_ge(spe, 1)
                vector.tensor_copy(h_sb[:, :], h_ps[:, :]).then_inc(scp, 1)
                vector.wait_ge(sh2, 32)
                vector.wait_ge(sTa, 16)
                for g in range(3):
                    o, i0, i1 = exp_aps(g)
                    vector.tensor_tensor(o, i0, i1, mybir.AluOpType.mult).then_inc(
                        stt, 1
                    )
                vector.wait_ge(sgp, 1)
                vector.wait_ge(sTb, 16)
                vector.tensor_scalar(
                    o_sb[3][:, :], tmp[3][:, :], b_sb[:, :], 0.0,
                    mybir.AluOpType.add, mybir.AluOpType.max,
                ).then_inc(sdv, 1)

            @block.gpsimd
            def _(gpsimd):
                gpsimd.wait_ge(sh2, 32)
                gpsimd.wait_ge(sTa, 16)
                o, i0, i1 = exp_aps(3)
                gpsimd.tensor_tensor(o, i0, i1, mybir.AluOpType.mult).then_inc(
                    sgp, 1
                )
                # all output data must have landed before teardown
                gpsimd.wait_ge(sout, 64)

        # teardown: barrier, then reset DMA state + semaphores for re-execution
        nums = sorted(s.num for s in sems)
        rng = range(nums[0], nums[-1] + 1)
        nc.all_engine_barrier()
        nc.gpsimd.dma_reset(rng)
        nc.gpsimd.sem_clear(rng)
        nc.all_engine_barrier()

    nc.compile()
    return nc


def _split_bf16(a):
    hi = a.astype(ml_dtypes.bfloat16)
    lo = (a - hi.astype(np.float32)).astype(ml_dtypes.bfloat16)
    return hi, lo


def _host_prepare(x, stft_w, conv_w, conv_b):
    """Build per-core input maps."""
    x = np.ascontiguousarray(x, dtype=np.float32)
    xp = np.zeros((B, C, 2 * Q + K), dtype=np.float32)  # padded to 4160
    xp[:, :, :S] = x
    sb_, sc_, ss_ = xp.strides
    win = np.lib.stride_tricks.as_strided(
        xp, shape=(B, C, Q, R), strides=(sb_, sc_, 2 * ss_, ss_)
    )
    Xf = win.reshape(B, C * Q, R)                      # (4, 4096, 65), b=2*pr+z
    # layout [p, kt, z, pr, r]: batch order (z,pr) -> b = [0, 2, 1, 3]
    X5 = np.ascontiguousarray(
        Xf[[0, 2, 1, 3]].reshape(2, 2, KT, 128, R).transpose(3, 2, 0, 1, 4)
    ).reshape(128, KT, W260)
    xh, xl = _split_bf16(X5)

    trow = np.ascontiguousarray(stft_w.T, dtype=np.float32).reshape(1, Q)

    in_maps = []
    for i in range(NCORES):
        cw_sh = conv_w[i * OSH : (i + 1) * OSH, :]     # (64, 4096)
        cwt = np.ascontiguousarray(
            cw_sh.reshape(OSH, KT, 128).transpose(2, 1, 0)  # (128, 32, 64)
        )
        ch, cl = _split_bf16(cwt)
        parts = [ch, cl, xh, xl] if PASSES == 3 else [ch, xh]
        wbuf = np.empty((128, NCH, NPASS_COLS * KTC), dtype=ml_dtypes.bfloat16)
        for chk in range(NCH):
            sl = slice(chk * KTC, (chk + 1) * KTC)
            wbuf[:, chk, :] = np.concatenate(
                [p_[:, sl].reshape(128, -1) for p_ in parts], axis=1
            )
        bias2 = np.ascontiguousarray(
            np.tile(conv_b[i * OSH : (i + 1) * OSH], 2).reshape(128, 1),
            dtype=np.float32,
        )
        in_maps.append(
            {
                "wbuf": wbuf.reshape(128, NCH * CHUNK_W),
                "trow": trow,
                "bias2": bias2,
            }
        )
    return in_maps


def kernel(x, stft_w, conv_w, conv_b):
    global _PROGRAM, _LAST_RESULTS
    if _PROGRAM is None:
        _PROGRAM = _build_program()
    in_maps = _host_prepare(
        np.asarray(x), np.asarray(stft_w, dtype=np.float32),
        np.asarray(conv_w, dtype=np.float32), np.asarray(conv_b, dtype=np.float32),
    )
    res = run_bass_kernel_spmd(_PROGRAM, in_maps, list(range(NCORES)))
    _LAST_RESULTS = res
    # per-core out: (256, 4096); rows pr*128 + z*64 + o' -> (b=2*pr+z, o=i*64+o')
    full = np.empty((B, OUT, S), dtype=np.float32)
    for i in range(NCORES):
        full[:, i * OSH : (i + 1) * OSH, :] = res.results[i]["out"].reshape(
            B, OSH, S
        )
    return full


if __name__ == "__main__":
    rng = np.random.default_rng(0)
    out = kernel(
        rng.standard_normal((B, C, S), dtype=np.float32),
        rng.standard_normal((NK, K), dtype=np.float32),
        (rng.standard_normal((OUT, C * K * NK)) * 0.02).astype(np.float32),
        (rng.standard_normal((OUT,)) * 0.02).astype(np.float32),
    )
    print(out.shape, out.dtype, float(np.abs(out).max()))


# revision 13
# speedup vs baseline: 1.1685x; 1.1521x over previous
"""Trainium2 Bass kernel for nn_CustomCNNLayer_84559316124470.

The reference computes, per batch b:
    win[b,c,s,m]   = xp[b,c,s+m]                    (xp = x padded with K-1 zeros)
    xw[b,c,s,m,l]  = win[b,c,s,m] * stft_w[l,m]
    xr             = xw.reshape(b, c*K*NK, s)       (raw row-major reshape)
    out            = relu(conv_w @ xr + bias)       (1x1 conv over channels)

Because K*NK == S/2 == 2048, the raw reshape maps
    xr[b, c*2048 + q, p*2048 + m*32 + l] = xp[b, c, 2q+p+m] * stft_w[l, m]
(with s = 2q+p). Hence, with h[b,o,r] = sum_{c,q} conv_w[o, c*2048+q] * xp[b,c,2q+r]
(r in [0, 65)):
    out[b, o, p*2048 + m*32 + l] = relu(stft_w[l,m] * h[b,o,p+m] + bias[o])

So the dense 8.6 GMAC/batch matmul collapses to a (512x4096)@(4096x65)
strided correlation (tensor engine) plus a per-element broadcast expansion
(vector/gpsimd engines) and bias+ReLU (scalar/vector engines).

Sharding: output channels o split across the 8 cores (64 rows each);
window matrices replicated. No collectives.

Precision: mm1 runs on the PE in bf16. With PASSES=3 the fp32 operands are
split hi/lo into bf16 pairs and three accumulating matmuls recover ~fp32
accuracy (error ~1e-5 rel.); fp32 PE matmuls run in multi-pass LOW_HIGH
mode and are not competitive.

Raw (non-Tile) implementation: hand-placed semaphores keep the setup and
teardown overhead minimal (Tile's vector-clock epilogue resets every
semaphore individually, ~10us).
"""

import numpy as np
import ml_dtypes

import concourse.bass as bass
from concourse import bacc, mybir
from concourse.bass_utils import run_bass_kernel_spmd

B, C, S = 4, 2, 4096
K, NK, OUT = 64, 32, 512
Q = K * NK            # 2048 == S // 2
R = K + 1             # 65 shift taps
NCORES = 8
OSH = OUT // NCORES   # 64 output channels per core
KT = 32               # contraction tiles of 128 over c*Q = 4096
W260 = B * R          # per-kt rhs free dim: [z=0 | z=1] x [pair 0 | pair 1] x r
PASSES = 3            # 1 = plain bf16, 3 = hi/lo split (near-fp32)
NCH = 4               # DMA chunks over kt
KTC = KT // NCH
NPASS_COLS = (2 * OSH + 2 * W260) if PASSES == 3 else (OSH + W260)
CHUNK_W = KTC * NPASS_COLS
F32 = mybir.dt.float32
BF16 = mybir.dt.bfloat16
GROUPS = [(0, 0), (0, 1), (1, 0), (1, 1)]  # (pr, p)

_PROGRAM = None
_LAST_RESULTS = None


def _build_program():
    nc = bacc.Bacc("TRN2", target_bir_lowering=False, debug=False)
    wbuf = nc.dram_tensor("wbuf", [128, NCH * CHUNK_W], BF16, kind="ExternalInput")
    trow = nc.dram_tensor("trow", [1, Q], F32, kind="ExternalInput")
    bias2 = nc.dram_tensor("bias2", [128, 1], F32, kind="ExternalInput")
    out = nc.dram_tensor("out", [2 * 128, S], F32, kind="ExternalOutput")

    co, xo = 0, (2 if PASSES == 3 else 1) * KTC * OSH
    pass_offs = [(co, xo)] if PASSES == 1 else [
        (co, xo),                      # ch @ xh
        (co, xo + KTC * W260),         # ch @ xl
        (co + KTC * OSH, xo),          # cl @ xh
    ]

    from contextlib import ExitStack

    with ExitStack() as ctx:
        e = ctx.enter_context
        wt = [e(nc.sbuf_tensor(f"wt{c}", [128, CHUNK_W], BF16)) for c in range(NCH)]
        T_sb = e(nc.sbuf_tensor("T_sb", [128, Q], F32))
        b_sb = e(nc.sbuf_tensor("b_sb", [128, 1], F32))
        h_sb = e(nc.sbuf_tensor("h_sb", [OSH, W260], F32))
        h2_sb = e(nc.sbuf_tensor("h2_sb", [128, 2 * R], F32))
        tmp = [e(nc.sbuf_tensor(f"tmp{g}", [128, Q], F32)) for g in range(4)]
        o_sb = [e(nc.sbuf_tensor(f"o{g}", [128, Q], F32)) for g in range(4)]
        h_ps = e(nc.psum_tensor("h_ps", [OSH, W260], F32))

        sin = [e(nc.semaphore(f"sin{c}")) for c in range(NCH)]  # chunk DMAs
        sTa = e(nc.semaphore("sTa"))    # T broadcast DMA
        sTb = e(nc.semaphore("sTb"))    # bias DMA
        spe = e(nc.semaphore("spe"))    # mm1 done
        scp = e(nc.semaphore("scp"))    # h psum->sbuf copy done
        sh2 = e(nc.semaphore("sh2"))    # h2 redistribution DMAs
        stt = e(nc.semaphore("stt"))    # DVE multiplies done (groups 0-2)
        sact = e(nc.semaphore("sact"))  # ACT relu done (groups 0-2)
        sdv = e(nc.semaphore("sdv"))    # DVE relu done (group 3)
        sout = e(nc.semaphore("sout"))  # out DMAs
        sems = sin + [sTa, sTb, spe, scp, sh2, stt, sact, sdv, sout]

        def exp_aps(g):
            pr, p = GROUPS[g]
            off = pr * R + p
            h_exp = (
                h2_sb[:, off : off + K].unsqueeze(2).to_broadcast((128, K, NK))
            )
            return (
                tmp[g][:, :].rearrange("a (m l) -> a m l", l=NK),
                h_exp,
                T_sb[:, :].rearrange("a (m l) -> a m l", l=NK),
            )

        with nc.Block() as block:

            @block.sync
            def _(sync):
                for c in range(NCH):
                    sync.dma_start(
                        wt[c][:, :], wbuf[:, c * CHUNK_W : (c + 1) * CHUNK_W]
                    ).then_inc(sin[c], 16)
                for z in range(2):
                    sync.wait_ge(scp, z + 1)
                    sync.dma_start(
                        h2_sb[z * OSH : (z + 1) * OSH, :],
                        h_sb[:, z * 2 * R : (z + 1) * 2 * R],
                    ).then_inc(sh2, 16)
                for g in range(4):
                    pr, p = GROUPS[g]
                    if g < 3:
                        sync.wait_ge(sact, g + 1)
                    else:
                        sync.wait_ge(sdv, 1)
                    sync.dma_start(
                        out[pr * 128 : (pr + 1) * 128, p * Q : (p + 1) * Q],
                        o_sb[g][:, :],
                    ).then_inc(sout, 16)

            @block.scalar
            def _(scalar):
                scalar.dma_start(
                    T_sb[:, :], trow[:, :].to_broadcast((128, Q))
                ).then_inc(sTa, 16)
                scalar.dma_start(b_sb[:, :], bias2[:, :]).then_inc(sTb, 16)
                scalar.wait_ge(sTb, 16)
                for g in range(3):
                    scalar.wait_ge(stt, g + 1)
                    scalar.activation(
                        o_sb[g][:, :], tmp[g][:, :],
                        mybir.ActivationFunctionType.Relu, bias=b_sb[:, :],
                    ).then_inc(sact, 1)

            @block.tensor
            def _(tensor):
                n_mm = NCH * KTC * len(pass_offs)
                i_mm = 0
                for chk in range(NCH):
                    tensor.wait_ge(sin[chk], 16)
                    for kt in range(KTC):
                        for c_off, x_off in pass_offs:
                            mm = tensor.matmul(
                                h_ps[:, :],
                                wt[chk][
                                    :, c_off + kt * OSH : c_off + (kt + 1) * OSH
                                ],
                                wt[chk][
                                    :, x_off + kt * W260 : x_off + (kt + 1) * W260
                                ],
                                start=(i_mm == 0),
                                stop=(i_mm == n_mm - 1),
                            )
                            i_mm += 1
                mm.then_inc(spe, 1)

            @block.vector
            def _(vector):
                vector.wait_ge(spe, 1)
                for z in range(2):
                    vector.tensor_copy(
                        h_sb[:, z * 2 * R : (z + 1) * 2 * R],
                        h_ps[:, z * 2 * R : (z + 1) * 2 * R],
                    ).then_inc(scp, 1)
                vector.wait_ge(sh2, 32)
                vector.wait_ge(sTa, 16)
                for g in range(4):
                    o, i0, i1 = exp_aps(g)
                    vector.tensor_tensor(o, i0, i1, mybir.AluOpType.mult).then_inc(
                        stt, 1
                    )
                vector.wait_ge(sTb, 16)
                vector.wait_ge(stt, 4)
                vector.tensor_scalar(
                    o_sb[3][:, :], tmp[3][:, :], b_sb[:, :], 0.0,
                    mybir.AluOpType.add, mybir.AluOpType.max,
                ).then_inc(sdv, 1)

    nc.compile()
    return nc


def _split_bf16(a):
    hi = a.astype(ml_dtypes.bfloat16)
    lo = (a - hi.astype(np.float32)).astype(ml_dtypes.bfloat16)
    return hi, lo


def _host_prepare(x, stft_w, conv_w, conv_b):
    """Build per-core input maps."""
    x = np.ascontiguousarray(x, dtype=np.float32)
    xp = np.zeros((B, C, 2 * Q + K), dtype=np.float32)  # padded to 4160
    xp[:, :, :S] = x
    sb_, sc_, ss_ = xp.strides
    win = np.lib.stride_tricks.as_strided(
        xp, shape=(B, C, Q, R), strides=(sb_, sc_, 2 * ss_, ss_)
    )
    Xf = win.reshape(B, C * Q, R)                      # (4, 4096, 65), b=2*pr+z
    # layout [p, kt, z, pr, r]: batch order (z,pr) -> b = [0, 2, 1, 3]
    X5 = np.ascontiguousarray(
        Xf[[0, 2, 1, 3]].reshape(2, 2, KT, 128, R).transpose(3, 2, 0, 1, 4)
    ).reshape(128, KT, W260)
    xh, xl = _split_bf16(X5)

    trow = np.ascontiguousarray(stft_w.T, dtype=np.float32).reshape(1, Q)

    in_maps = []
    for i in range(NCORES):
        cw_sh = conv_w[i * OSH : (i + 1) * OSH, :]     # (64, 4096)
        cwt = np.ascontiguousarray(
            cw_sh.reshape(OSH, KT, 128).transpose(2, 1, 0)  # (128, 32, 64)
        )
        ch, cl = _split_bf16(cwt)
        parts = [ch, cl, xh, xl] if PASSES == 3 else [ch, xh]
        wbuf = np.empty((128, NCH, NPASS_COLS * KTC), dtype=ml_dtypes.bfloat16)
        for chk in range(NCH):
            sl = slice(chk * KTC, (chk + 1) * KTC)
            wbuf[:, chk, :] = np.concatenate(
                [p_[:, sl].reshape(128, -1) for p_ in parts], axis=1
            )
        bias2 = np.ascontiguousarray(
            np.tile(conv_b[i * OSH : (i + 1) * OSH], 2).reshape(128, 1),
            dtype=np.float32,
        )
        in_maps.append(
            {
                "wbuf": wbuf.reshape(128, NCH * CHUNK_W),
                "trow": trow,
                "bias2": bias2,
            }
        )
    return in_maps


def kernel(x, stft_w, conv_w, conv_b):
    global _PROGRAM, _LAST_RESULTS
    if _PROGRAM is None:
        _PROGRAM = _build_program()
    in_maps = _host_prepare(
        np.asarray(x), np.asarray(stft_w, dtype=np.float32),
        np.asarray(conv_w, dtype=np.float32), np.asarray(conv_b, dtype=np.float32),
    )
    res = run_bass_kernel_spmd(_PROGRAM, in_maps, list(range(NCORES)))
    _LAST_RESULTS = res
    # per-core out: (256, 4096); rows pr*128 + z*64 + o' -> (b=2*pr+z, o=i*64+o')
    full = np.empty((B, OUT, S), dtype=np.float32)
    for i in range(NCORES):
        full[:, i * OSH : (i + 1) * OSH, :] = res.results[i]["out"].reshape(
            B, OSH, S
        )
    return full


if __name__ == "__main__":
    rng = np.random.default_rng(0)
    out = kernel(
        rng.standard_normal((B, C, S), dtype=np.float32),
        rng.standard_normal((NK, K), dtype=np.float32),
        (rng.standard_normal((OUT, C * K * NK)) * 0.02).astype(np.float32),
        (rng.standard_normal((OUT,)) * 0.02).astype(np.float32),
    )
    print(out.shape, out.dtype, float(np.abs(out).max()))


# revision 16
# speedup vs baseline: 1.2226x; 1.0463x over previous
"""Trainium2 Bass kernel for nn_CustomCNNLayer_84559316124470.

The reference computes, per batch b:
    win[b,c,s,m]   = xp[b,c,s+m]                    (xp = x padded with K-1 zeros)
    xw[b,c,s,m,l]  = win[b,c,s,m] * stft_w[l,m]
    xr             = xw.reshape(b, c*K*NK, s)       (raw row-major reshape)
    out            = relu(conv_w @ xr + bias)       (1x1 conv over channels)

Because K*NK == S/2 == 2048, the raw reshape maps
    xr[b, c*2048 + q, p*2048 + m*32 + l] = xp[b, c, 2q+p+m] * stft_w[l, m]
(with s = 2q+p). Hence, with h[b,o,r] = sum_{c,q} conv_w[o, c*2048+q] * xp[b,c,2q+r]
(r in [0, 65)):
    out[b, o, p*2048 + m*32 + l] = relu(stft_w[l,m] * h[b,o,p+m] + bias[o])

So the dense 8.6 GMAC/batch matmul collapses to a (512x4096)@(4096x65)
strided correlation (tensor engine) plus a per-element broadcast expansion
(vector/gpsimd engines) and bias+ReLU (scalar/vector engines).

Sharding: output channels o split across the 8 cores (64 rows each);
window matrices replicated. No collectives.

Precision: mm1 runs on the PE in bf16. With PASSES=3 the fp32 operands are
split hi/lo into bf16 pairs and three accumulating matmuls recover ~fp32
accuracy (error ~1e-5 rel.); fp32 PE matmuls run in multi-pass LOW_HIGH
mode and are not competitive.

Raw (non-Tile) implementation: hand-placed semaphores keep the setup and
teardown overhead minimal (Tile's vector-clock epilogue resets every
semaphore individually, ~10us).
"""

import numpy as np
import ml_dtypes

import concourse.bass as bass
from concourse import bacc, mybir
from concourse.bass_utils import run_bass_kernel_spmd

B, C, S = 4, 2, 4096
K, NK, OUT = 64, 32, 512
Q = K * NK            # 2048 == S // 2
R = K + 1             # 65 shift taps
NCORES = 8
OSH = OUT // NCORES   # 64 output channels per core
KT = 32               # contraction tiles of 128 over c*Q = 4096
W260 = B * R          # per-kt rhs free dim: [z=0 | z=1] x [pair 0 | pair 1] x r
PASSES = 3            # 1 = plain bf16, 3 = hi/lo split (near-fp32)
NCH = 4               # DMA chunks over kt
KTC = KT // NCH
NPASS_COLS = (2 * OSH + 2 * W260) if PASSES == 3 else (OSH + W260)
CHUNK_W = KTC * NPASS_COLS
F32 = mybir.dt.float32
BF16 = mybir.dt.bfloat16
GROUPS = [(0, 0), (0, 1), (1, 0), (1, 1)]  # (pr, p)

_PROGRAM = None
_LAST_RESULTS = None


def _build_program():
    nc = bacc.Bacc("TRN2", target_bir_lowering=False, debug=False)
    wbuf = nc.dram_tensor("wbuf", [128, NCH * CHUNK_W], BF16, kind="ExternalInput")
    trow = nc.dram_tensor("trow", [1, Q], F32, kind="ExternalInput")
    bias2 = nc.dram_tensor("bias2", [128, 1], F32, kind="ExternalInput")
    ident = nc.dram_tensor("ident", [OSH, OSH], F32, kind="ExternalInput")
    out = nc.dram_tensor("out", [2 * 128, S], F32, kind="ExternalOutput")

    co, xo = 0, (2 if PASSES == 3 else 1) * KTC * OSH
    pass_offs = [(co, xo)] if PASSES == 1 else [
        (co, xo),                      # ch @ xh
        (co, xo + KTC * W260),         # ch @ xl
        (co + KTC * OSH, xo),          # cl @ xh
    ]

    from contextlib import ExitStack

    with ExitStack() as ctx:
        e = ctx.enter_context
        wt = [e(nc.sbuf_tensor(f"wt{c}", [128, CHUNK_W], BF16)) for c in range(NCH)]
        T_sb = e(nc.sbuf_tensor("T_sb", [128, Q], F32))
        b_sb = e(nc.sbuf_tensor("b_sb", [128, 1], F32))
        h_sb = e(nc.sbuf_tensor("h_sb", [OSH, W260], F32))
        id_sb = e(nc.sbuf_tensor("id_sb", [OSH, OSH], F32))
        tmp = [e(nc.sbuf_tensor(f"tmp{g}", [128, Q], F32)) for g in range(4)]
        o_sb = [e(nc.sbuf_tensor(f"o{g}", [128, Q], F32)) for g in range(4)]
        # pad h_ps to a full 2KB PSUM bank so h2_ps lands in its own bank
        h_ps_full = e(nc.psum_tensor("h_ps", [OSH, 512], F32))
        h_ps = h_ps_full[:, :W260]
        h2_ps_full = e(nc.psum_tensor("h2_ps", [128, 512], F32))
        h2_ps = h2_ps_full[:, : 2 * R]

        sin = [e(nc.semaphore(f"sin{c}")) for c in range(NCH)]  # chunk DMAs
        sTa = e(nc.semaphore("sTa"))    # T broadcast DMA
        sTb = e(nc.semaphore("sTb"))    # bias DMA
        sid = e(nc.semaphore("sid"))    # identity DMA
        spe = e(nc.semaphore("spe"))    # mm1 done
        scp = e(nc.semaphore("scp"))    # h psum->sbuf copy done
        sh2 = e(nc.semaphore("sh2"))    # h2 redistribution DMAs
        stt = e(nc.semaphore("stt"))    # DVE multiplies done (groups 0-2)
        sact = e(nc.semaphore("sact"))  # ACT relu done (groups 0-2)
        sdv = e(nc.semaphore("sdv"))    # DVE relu done (group 3)
        sout = e(nc.semaphore("sout"))  # out DMAs
        sems = sin + [sTa, sTb, sid, spe, scp, sh2, stt, sact, sdv, sout]

        def exp_aps_half(g, hf):
            pr, p = GROUPS[g]
            off = pr * R + p + hf * (K // 2)
            h_exp = (
                h2_ps[:, off : off + K // 2]
                .unsqueeze(2)
                .to_broadcast((128, K // 2, NK))
            )
            sl = bass.ts(hf, Q // 2)
            return (
                tmp[g][:, sl].rearrange("a (m l) -> a m l", l=NK),
                h_exp,
                T_sb[:, sl].rearrange("a (m l) -> a m l", l=NK),
            )

        with nc.Block() as block:

            @block.sync
            def _(sync):
                for c in range(NCH):
                    sync.dma_start(
                        wt[c][:, :], wbuf[:, c * CHUNK_W : (c + 1) * CHUNK_W]
                    ).then_inc(sin[c], 16)
                # out DMAs: half-width units for earlier streaming
                n_act = 0
                n_dv = 0
                for g in range(4):
                    pr, p = GROUPS[g]
                    for hf in range(2):
                        if g < 3:
                            n_act += 1
                            sync.wait_ge(sact, n_act)
                        else:
                            n_dv += 1
                            sync.wait_ge(sdv, n_dv)
                        sync.dma_start(
                            out[
                                pr * 128 : (pr + 1) * 128,
                                p * Q + hf * (Q // 2) : p * Q + (hf + 1) * (Q // 2),
                            ],
                            o_sb[g][:, bass.ts(hf, Q // 2)],
                        ).then_inc(sout, 16)

            @block.scalar
            def _(scalar):
                scalar.dma_start(b_sb[:, :], bias2[:, :]).then_inc(sTb, 16)
                scalar.dma_start(id_sb[:, :], ident[:, :]).then_inc(sid, 16)
                # delay the 1MB T broadcast until chunk0 has landed so mm1
                # starts as early as possible
                scalar.wait_ge(sin[0], 16)
                scalar.dma_start(
                    T_sb[:, :], trow[:, :].to_broadcast((128, Q))
                ).then_inc(sTa, 16)
                scalar.wait_ge(sTb, 16)
                n_tt = 0
                n_act = 0
                for g in range(3):
                    for hf in range(2):
                        n_tt += 1
                        scalar.wait_ge(stt, n_tt)
                        n_act += 1
                        scalar.activation(
                            o_sb[g][:, bass.ts(hf, Q // 2)],
                            tmp[g][:, bass.ts(hf, Q // 2)],
                            mybir.ActivationFunctionType.Relu, bias=b_sb[:, :],
                        ).then_inc(sact, 1)

            @block.tensor
            def _(tensor):
                n_mm = NCH * KTC * len(pass_offs)
                i_mm = 0
                for chk in range(NCH):
                    tensor.wait_ge(sin[chk], 16)
                    for kt in range(KTC):
                        for c_off, x_off in pass_offs:
                            mm = tensor.matmul(
                                h_ps[:, :],
                                wt[chk][
                                    :, c_off + kt * OSH : c_off + (kt + 1) * OSH
                                ],
                                wt[chk][
                                    :, x_off + kt * W260 : x_off + (kt + 1) * W260
                                ],
                                start=(i_mm == 0),
                                stop=(i_mm == n_mm - 1),
                            )
                            if i_mm == n_mm - 1:
                                mm.then_inc(spe, 1)
                            i_mm += 1
                # redistribute h (64, [z|pr|r]) -> h2 (z*64+o', pr*65+r) with
                # identity matmuls on the (already warm) PE: no DMA receipt.
                tensor.wait_ge(sid, 16)
                for z in range(2):
                    tensor.wait_ge(scp, z + 1)
                    tensor.matmul(
                        h2_ps[z * OSH : (z + 1) * OSH, :],
                        id_sb[:, :],
                        h_sb[:, z * 2 * R : (z + 1) * 2 * R],
                        start=True,
                        stop=True,
                    ).then_inc(sh2, 1)

            @block.vector
            def _(vector):
                vector.wait_ge(spe, 1)
                for z in range(2):
                    vector.tensor_copy(
                        h_sb[:, z * 2 * R : (z + 1) * 2 * R],
                        h_ps[:, z * 2 * R : (z + 1) * 2 * R],
                    ).then_inc(scp, 1)
                vector.wait_ge(sh2, 2)
                vector.wait_ge(sTa, 16)
                n_tt = 0
                for g in range(4):
                    for hf in range(2):
                        o, i0, i1 = exp_aps_half(g, hf)
                        tt = vector.tensor_tensor(
                            o, i0, i1, mybir.AluOpType.mult
                        ).then_inc(stt, 1)
                        n_tt += 1
                        if g == 3:
                            if hf == 0:
                                vector.wait_ge(sTb, 16)
                            vector.wait_ge(stt, n_tt)
                            vector.tensor_scalar(
                                o_sb[3][:, bass.ts(hf, Q // 2)],
                                tmp[3][:, bass.ts(hf, Q // 2)],
                                b_sb[:, :], 0.0,
                                mybir.AluOpType.add, mybir.AluOpType.max,
                            ).then_inc(sdv, 1)

    nc.compile()
    return nc


def _split_bf16(a):
    hi = a.astype(ml_dtypes.bfloat16)
    lo = (a - hi.astype(np.float32)).astype(ml_dtypes.bfloat16)
    return hi, lo


def _host_prepare(x, stft_w, conv_w, conv_b):
    """Build per-core input maps."""
    x = np.ascontiguousarray(x, dtype=np.float32)
    xp = np.zeros((B, C, 2 * Q + K), dtype=np.float32)  # padded to 4160
    xp[:, :, :S] = x
    sb_, sc_, ss_ = xp.strides
    win = np.lib.stride_tricks.as_strided(
        xp, shape=(B, C, Q, R), strides=(sb_, sc_, 2 * ss_, ss_)
    )
    Xf = win.reshape(B, C * Q, R)                      # (4, 4096, 65), b=2*pr+z
    # layout [p, kt, z, pr, r]: batch order (z,pr) -> b = [0, 2, 1, 3]
    X5 = np.ascontiguousarray(
        Xf[[0, 2, 1, 3]].reshape(2, 2, KT, 128, R).transpose(3, 2, 0, 1, 4)
    ).reshape(128, KT, W260)
    xh, xl = _split_bf16(X5)

    trow = np.ascontiguousarray(stft_w.T, dtype=np.float32).reshape(1, Q)

    in_maps = []
    for i in range(NCORES):
        cw_sh = conv_w[i * OSH : (i + 1) * OSH, :]     # (64, 4096)
        cwt = np.ascontiguousarray(
            cw_sh.reshape(OSH, KT, 128).transpose(2, 1, 0)  # (128, 32, 64)
        )
        ch, cl = _split_bf16(cwt)
        parts = [ch, cl, xh, xl] if PASSES == 3 else [ch, xh]
        wbuf = np.empty((128, NCH, NPASS_COLS * KTC), dtype=ml_dtypes.bfloat16)
        for chk in range(NCH):
            sl = slice(chk * KTC, (chk + 1) * KTC)
            wbuf[:, chk, :] = np.concatenate(
                [p_[:, sl].reshape(128, -1) for p_ in parts], axis=1
            )
        bias2 = np.ascontiguousarray(
            np.tile(conv_b[i * OSH : (i + 1) * OSH], 2).reshape(128, 1),
            dtype=np.float32,
        )
        in_maps.append(
            {
                "wbuf": wbuf.reshape(128, NCH * CHUNK_W),
                "trow": trow,
                "bias2": bias2,
                "ident": np.eye(OSH, dtype=np.float32),
            }
        )
    return in_maps


def kernel(x, stft_w, conv_w, conv_b):
    global _PROGRAM, _LAST_RESULTS
    if _PROGRAM is None:
        _PROGRAM = _build_program()
    in_maps = _host_prepare(
        np.asarray(x), np.asarray(stft_w, dtype=np.float32),
        np.asarray(conv_w, dtype=np.float32), np.asarray(conv_b, dtype=np.float32),
    )
    res = run_bass_kernel_spmd(_PROGRAM, in_maps, list(range(NCORES)))
    _LAST_RESULTS = res
    # per-core out: (256, 4096); rows pr*128 + z*64 + o' -> (b=2*pr+z, o=i*64+o')
    full = np.empty((B, OUT, S), dtype=np.float32)
    for i in range(NCORES):
        full[:, i * OSH : (i + 1) * OSH, :] = res.results[i]["out"].reshape(
            B, OSH, S
        )
    return full


if __name__ == "__main__":
    rng = np.random.default_rng(0)
    out = kernel(
        rng.standard_normal((B, C, S), dtype=np.float32),
        rng.standard_normal((NK, K), dtype=np.float32),
        (rng.standard_normal((OUT, C * K * NK)) * 0.02).astype(np.float32),
        (rng.standard_normal((OUT,)) * 0.02).astype(np.float32),
    )
    print(out.shape, out.dtype, float(np.abs(out).max()))


# revision 19
# speedup vs baseline: 1.2255x; 1.0023x over previous
"""Trainium2 Bass kernel for nn_CustomCNNLayer_84559316124470.

The reference computes, per batch b:
    win[b,c,s,m]   = xp[b,c,s+m]                    (xp = x padded with K-1 zeros)
    xw[b,c,s,m,l]  = win[b,c,s,m] * stft_w[l,m]
    xr             = xw.reshape(b, c*K*NK, s)       (raw row-major reshape)
    out            = relu(conv_w @ xr + bias)       (1x1 conv over channels)

Because K*NK == S/2 == 2048, the raw reshape maps
    xr[b, c*2048 + q, p*2048 + m*32 + l] = xp[b, c, 2q+p+m] * stft_w[l, m]
(with s = 2q+p). Hence, with h[b,o,r] = sum_{c,q} conv_w[o, c*2048+q] * xp[b,c,2q+r]
(r in [0, 65)):
    out[b, o, p*2048 + m*32 + l] = relu(stft_w[l,m] * h[b,o,p+m] + bias[o])

So the dense 8.6 GMAC/batch matmul collapses to a (512x4096)@(4096x65)
strided correlation (tensor engine) plus a per-element broadcast expansion
(vector/gpsimd engines) and bias+ReLU (scalar/vector engines).

Sharding: output channels o split across the 8 cores (64 rows each);
window matrices replicated. No collectives.

Precision: mm1 runs on the PE in bf16. With PASSES=3 the fp32 operands are
split hi/lo into bf16 pairs and three accumulating matmuls recover ~fp32
accuracy (error ~1e-5 rel.); fp32 PE matmuls run in multi-pass LOW_HIGH
mode and are not competitive.

Raw (non-Tile) implementation: hand-placed semaphores keep the setup and
teardown overhead minimal (Tile's vector-clock epilogue resets every
semaphore individually, ~10us).
"""

import numpy as np
import ml_dtypes

import concourse.bass as bass
from concourse import bacc, mybir
from concourse.bass_utils import run_bass_kernel_spmd

B, C, S = 4, 2, 4096
K, NK, OUT = 64, 32, 512
Q = K * NK            # 2048 == S // 2
R = K + 1             # 65 shift taps
NCORES = 8
OSH = OUT // NCORES   # 64 output channels per core
KT = 32               # contraction tiles of 128 over c*Q = 4096
W260 = B * R          # per-kt rhs free dim: [z=0 | z=1] x [pair 0 | pair 1] x r
NCH = 4               # DMA chunks over kt
KTC = KT // NCH
CHUNK_W = KTC * (2 * OSH + W260)    # bf16 stream: [ch | cl | xh]
CHUNK8_W = KTC * (OSH + W260)       # fp8 stream:  [c8 | xl9]
CSC, XSC = 16.0, 512.0              # fp8 encode scales; product scale 2^13
F32 = mybir.dt.float32
BF16 = mybir.dt.bfloat16
F8 = mybir.dt.float8e4
GROUPS = [(0, 0), (0, 1), (1, 0), (1, 1)]  # (pr, p)

_PROGRAM = None
_LAST_RESULTS = None


def _build_program():
    nc = bacc.Bacc("TRN2", target_bir_lowering=False, debug=False)
    wbuf = nc.dram_tensor("wbuf", [128, NCH * CHUNK_W], BF16, kind="ExternalInput")
    # fp8 payload travels as uint8 (PJRT lacks float8_e4m3 support) and is
    # bitcast to fp8 at the matmul APs
    wbuf8 = nc.dram_tensor(
        "wbuf8", [128, NCH * CHUNK8_W], mybir.dt.uint8, kind="ExternalInput"
    )
    trow = nc.dram_tensor("trow", [1, Q], F32, kind="ExternalInput")
    bias2 = nc.dram_tensor("bias2", [128, 1], F32, kind="ExternalInput")
    ident = nc.dram_tensor("ident", [OSH, OSH], F32, kind="ExternalInput")
    out = nc.dram_tensor("out", [2 * 128, S], F32, kind="ExternalOutput")

    from contextlib import ExitStack

    with ExitStack() as ctx:
        e = ctx.enter_context
        wt = [e(nc.sbuf_tensor(f"wt{c}", [128, CHUNK_W], BF16)) for c in range(NCH)]
        wt8 = [
            e(nc.sbuf_tensor(f"wt8{c}", [128, CHUNK8_W], mybir.dt.uint8))
            for c in range(NCH)
        ]
        T_sb = e(nc.sbuf_tensor("T_sb", [128, Q], F32))
        b_sb = e(nc.sbuf_tensor("b_sb", [128, 1], F32))
        h_sb = e(nc.sbuf_tensor("h_sb", [OSH, W260], F32))
        id_sb = e(nc.sbuf_tensor("id_sb", [OSH, OSH], F32))
        tmp = [e(nc.sbuf_tensor(f"tmp{g}", [128, Q], F32)) for g in range(4)]
        o_sb = [e(nc.sbuf_tensor(f"o{g}", [128, Q], F32)) for g in range(4)]
        # pad h_ps to a full 2KB PSUM bank so h2_ps lands in its own bank
        h_ps_full = e(nc.psum_tensor("h_ps", [OSH, 512], F32))
        h_ps = h_ps_full[:, :W260]
        corr_ps_full = e(nc.psum_tensor("corr_ps", [OSH, 512], F32))
        corr_ps = corr_ps_full[:, :W260]
        h2_ps_full = e(nc.psum_tensor("h2_ps", [128, 512], F32))
        h2_ps = h2_ps_full[:, : 2 * R]

        sin = [e(nc.semaphore(f"sin{c}")) for c in range(NCH)]  # bf16 chunk DMAs
        sin8 = [e(nc.semaphore(f"si8{c}")) for c in range(NCH)]  # fp8 chunk DMAs
        sTa = e(nc.semaphore("sTa"))    # T broadcast DMA
        sTb = e(nc.semaphore("sTb"))    # bias DMA
        sid = e(nc.semaphore("sid"))    # identity DMA
        spe = e(nc.semaphore("spe"))    # mm1 done
        scp = e(nc.semaphore("scp"))    # h combine done
        scpa = e(nc.semaphore("scpa"))  # h psum->sbuf copy done
        sh2 = e(nc.semaphore("sh2"))    # h2 redistribution DMAs
        stt = e(nc.semaphore("stt"))    # DVE multiplies done (groups 0-2)
        sact = e(nc.semaphore("sact"))  # ACT relu done (groups 0-2)
        sdv = e(nc.semaphore("sdv"))    # DVE relu done (group 3)
        sout = e(nc.semaphore("sout"))  # out DMAs
        sems = sin + sin8 + [sTa, sTb, sid, spe, scp, scpa, sh2, stt, sact, sdv, sout]

        def exp_aps_half(g, hf):
            pr, p = GROUPS[g]
            off = pr * R + p + hf * (K // 2)
            h_exp = (
                h2_ps[:, off : off + K // 2]
                .unsqueeze(2)
                .to_broadcast((128, K // 2, NK))
            )
            sl = bass.ts(hf, Q // 2)
            return (
                tmp[g][:, sl].rearrange("a (m l) -> a m l", l=NK),
                h_exp,
                T_sb[:, sl].rearrange("a (m l) -> a m l", l=NK),
            )

        with nc.Block() as block:

            @block.sync
            def _(sync):
                for c in range(NCH):
                    sync.dma_start(
                        wt[c][:, :], wbuf[:, c * CHUNK_W : (c + 1) * CHUNK_W]
                    ).then_inc(sin[c], 16)
                # out DMAs: half-width units for earlier streaming
                n_act = 0
                n_dv = 0
                for g in range(4):
                    pr, p = GROUPS[g]
                    for hf in range(2):
                        if g < 3:
                            n_act += 1
                            sync.wait_ge(sact, n_act)
                        else:
                            n_dv += 1
                            sync.wait_ge(sdv, n_dv)
                        sync.dma_start(
                            out[
                                pr * 128 : (pr + 1) * 128,
                                p * Q + hf * (Q // 2) : p * Q + (hf + 1) * (Q // 2),
                            ],
                            o_sb[g][:, bass.ts(hf, Q // 2)],
                        ).then_inc(sout, 16)

            @block.scalar
            def _(scalar):
                scalar.dma_start(b_sb[:, :], bias2[:, :]).then_inc(sTb, 16)
                scalar.dma_start(id_sb[:, :], ident[:, :]).then_inc(sid, 16)
                for c in range(NCH):
                    scalar.dma_start(
                        wt8[c][:, :], wbuf8[:, c * CHUNK8_W : (c + 1) * CHUNK8_W]
                    ).then_inc(sin8[c], 16)
                scalar.dma_start(
                    T_sb[:, :], trow[:, :].to_broadcast((128, Q))
                ).then_inc(sTa, 16)
                scalar.wait_ge(sTb, 16)
                n_tt = 0
                n_act = 0
                for g in range(3):
                    for hf in range(2):
                        n_tt += 1
                        scalar.wait_ge(stt, n_tt)
                        n_act += 1
                        scalar.activation(
                            o_sb[g][:, bass.ts(hf, Q // 2)],
                            tmp[g][:, bass.ts(hf, Q // 2)],
                            mybir.ActivationFunctionType.Relu, bias=b_sb[:, :],
                        ).then_inc(sact, 1)

            @block.tensor
            def _(tensor):
                xo = 2 * KTC * OSH          # xh offset in bf16 chunk
                x8o = KTC * OSH             # xl9 offset in fp8 chunk
                n_main = NCH * KTC * 2
                n_corr = NCH * KTC
                i_main = i_corr = 0
                for chk in range(NCH):
                    tensor.wait_ge(sin[chk], 16)
                    tensor.wait_ge(sin8[chk], 16)
                    for kt in range(KTC):
                        xh_t = wt[chk][:, xo + kt * W260 : xo + (kt + 1) * W260]
                        for c_off in (0, KTC * OSH):
                            tensor.matmul(
                                h_ps[:, :],
                                wt[chk][
                                    :, c_off + kt * OSH : c_off + (kt + 1) * OSH
                                ],
                                xh_t,
                                start=(i_main == 0),
                                stop=(i_main == n_main - 1),
                            )
                            i_main += 1
                        mm = tensor.matmul(
                            corr_ps[:, :],
                            wt8[chk][:, kt * OSH : (kt + 1) * OSH].bitcast(F8),
                            wt8[chk][
                                :, x8o + kt * W260 : x8o + (kt + 1) * W260
                            ].bitcast(F8),
                            start=(i_corr == 0),
                            stop=(i_corr == n_corr - 1),
                        )
                        if i_corr == n_corr - 1:
                            mm.then_inc(spe, 1)
                        i_corr += 1
                # redistribute h (64, [z|pr|r]) -> h2 (z*64+o', pr*65+r) with
                # identity matmuls on the (already warm) PE: no DMA receipt.
                tensor.wait_ge(sid, 16)
                for z in range(2):
                    tensor.wait_ge(scp, z + 1)
                    tensor.matmul(
                        h2_ps[z * OSH : (z + 1) * OSH, :],
                        id_sb[:, :],
                        h_sb[:, z * 2 * R : (z + 1) * 2 * R],
                        start=True,
                        stop=True,
                    ).then_inc(sh2, 1)

            @block.vector
            def _(vector):
                vector.wait_ge(spe, 1)
                for z in range(2):
                    sl = slice(z * 2 * R, (z + 1) * 2 * R)
                    vector.tensor_copy(h_sb[:, sl], h_ps[:, sl]).then_inc(scpa, 1)
                for z in range(2):
                    sl = slice(z * 2 * R, (z + 1) * 2 * R)
                    vector.wait_ge(scpa, z + 1)
                    vector.scalar_tensor_tensor(
                        h_sb[:, sl], corr_ps[:, sl], 2.0 ** -13, h_sb[:, sl],
                        mybir.AluOpType.mult, mybir.AluOpType.add,
                    ).then_inc(scp, 1)
                vector.wait_ge(sh2, 2)
                vector.wait_ge(sTa, 16)
                n_tt = 0
                for g in range(4):
                    for hf in range(2):
                        o, i0, i1 = exp_aps_half(g, hf)
                        tt = vector.tensor_tensor(
                            o, i0, i1, mybir.AluOpType.mult
                        ).then_inc(stt, 1)
                        n_tt += 1
                        if g == 3:
                            if hf == 0:
                                vector.wait_ge(sTb, 16)
                            vector.wait_ge(stt, n_tt)
                            vector.tensor_scalar(
                                o_sb[3][:, bass.ts(hf, Q // 2)],
                                tmp[3][:, bass.ts(hf, Q // 2)],
                                b_sb[:, :], 0.0,
                                mybir.AluOpType.add, mybir.AluOpType.max,
                            ).then_inc(sdv, 1)

    nc.compile()
    return nc


def _split_bf16(a):
    hi = a.astype(ml_dtypes.bfloat16)
    lo = (a - hi.astype(np.float32)).astype(ml_dtypes.bfloat16)
    return hi, lo


def _host_prepare(x, stft_w, conv_w, conv_b):
    """Build per-core input maps."""
    x = np.ascontiguousarray(x, dtype=np.float32)
    xp = np.zeros((B, C, 2 * Q + K), dtype=np.float32)  # padded to 4160
    xp[:, :, :S] = x
    sb_, sc_, ss_ = xp.strides
    win = np.lib.stride_tricks.as_strided(
        xp, shape=(B, C, Q, R), strides=(sb_, sc_, 2 * ss_, ss_)
    )
    Xf = win.reshape(B, C * Q, R)                      # (4, 4096, 65), b=2*pr+z
    # layout [p, kt, z, pr, r]: batch order (z,pr) -> b = [0, 2, 1, 3]
    X5 = np.ascontiguousarray(
        Xf[[0, 2, 1, 3]].reshape(2, 2, KT, 128, R).transpose(3, 2, 0, 1, 4)
    ).reshape(128, KT, W260)
    xh, xl = _split_bf16(X5)
    xl9 = np.ascontiguousarray(
        (xl.astype(np.float32) * XSC)
    ).astype(ml_dtypes.float8_e4m3)

    trow = np.ascontiguousarray(stft_w.T, dtype=np.float32).reshape(1, Q)

    in_maps = []
    for i in range(NCORES):
        cw_sh = conv_w[i * OSH : (i + 1) * OSH, :]     # (64, 4096)
        cwt = np.ascontiguousarray(
            cw_sh.reshape(OSH, KT, 128).transpose(2, 1, 0)  # (128, 32, 64)
        )
        ch, cl = _split_bf16(cwt)
        c8 = (ch.astype(np.float32) * CSC).astype(ml_dtypes.float8_e4m3)
        wbuf = np.empty((128, NCH, CHUNK_W), dtype=ml_dtypes.bfloat16)
        wbuf8 = np.empty((128, NCH, CHUNK8_W), dtype=ml_dtypes.float8_e4m3)
        for chk in range(NCH):
            sl = slice(chk * KTC, (chk + 1) * KTC)
            wbuf[:, chk, :] = np.concatenate(
                [p_[:, sl].reshape(128, -1) for p_ in (ch, cl, xh)], axis=1
            )
            wbuf8[:, chk, :] = np.concatenate(
                [p_[:, sl].reshape(128, -1) for p_ in (c8, xl9)], axis=1
            )
        bias2 = np.ascontiguousarray(
            np.tile(conv_b[i * OSH : (i + 1) * OSH], 2).reshape(128, 1),
            dtype=np.float32,
        )
        in_maps.append(
            {
                "wbuf": wbuf.reshape(128, NCH * CHUNK_W),
                "wbuf8": wbuf8.reshape(128, NCH * CHUNK8_W).view(np.uint8),
                "trow": trow,
                "bias2": bias2,
                "ident": np.eye(OSH, dtype=np.float32),
            }
        )
    return in_maps


def kernel(x, stft_w, conv_w, conv_b):
    global _PROGRAM, _LAST_RESULTS
    if _PROGRAM is None:
        _PROGRAM = _build_program()
    in_maps = _host_prepare(
        np.asarray(x), np.asarray(stft_w, dtype=np.float32),
        np.asarray(conv_w, dtype=np.float32), np.asarray(conv_b, dtype=np.float32),
    )
    res = run_bass_kernel_spmd(_PROGRAM, in_maps, list(range(NCORES)))
    _LAST_RESULTS = res
    # per-core out: (256, 4096); rows pr*128 + z*64 + o' -> (b=2*pr+z, o=i*64+o')
    full = np.empty((B, OUT, S), dtype=np.float32)
    for i in range(NCORES):
        full[:, i * OSH : (i + 1) * OSH, :] = res.results[i]["out"].reshape(
            B, OSH, S
        )
    return full


if __name__ == "__main__":
    rng = np.random.default_rng(0)
    out = kernel(
        rng.standard_normal((B, C, S), dtype=np.float32),
        rng.standard_normal((NK, K), dtype=np.float32),
        (rng.standard_normal((OUT, C * K * NK)) * 0.02).astype(np.float32),
        (rng.standard_normal((OUT,)) * 0.02).astype(np.float32),
    )
    print(out.shape, out.dtype, float(np.abs(out).max()))


# revision 20
# speedup vs baseline: 1.2679x; 1.0346x over previous
"""Trainium2 Bass kernel for nn_CustomCNNLayer_84559316124470.

The reference computes, per batch b:
    win[b,c,s,m]   = xp[b,c,s+m]                    (xp = x padded with K-1 zeros)
    xw[b,c,s,m,l]  = win[b,c,s,m] * stft_w[l,m]
    xr             = xw.reshape(b, c*K*NK, s)       (raw row-major reshape)
    out            = relu(conv_w @ xr + bias)       (1x1 conv over channels)

Because K*NK == S/2 == 2048, the raw reshape maps
    xr[b, c*2048 + q, p*2048 + m*32 + l] = xp[b, c, 2q+p+m] * stft_w[l, m]
(with s = 2q+p). Hence, with h[b,o,r] = sum_{c,q} conv_w[o, c*2048+q] * xp[b,c,2q+r]
(r in [0, 65)):
    out[b, o, p*2048 + m*32 + l] = relu(stft_w[l,m] * h[b,o,p+m] + bias[o])

So the dense 8.6 GMAC/batch matmul collapses to a (512x4096)@(4096x65)
strided correlation (tensor engine) plus a per-element broadcast expansion
(vector/gpsimd engines) and bias+ReLU (scalar/vector engines).

Sharding: output channels o split across the 8 cores (64 rows each);
window matrices replicated. No collectives.

Precision: mm1 runs on the PE in bf16. With PASSES=3 the fp32 operands are
split hi/lo into bf16 pairs and three accumulating matmuls recover ~fp32
accuracy (error ~1e-5 rel.); fp32 PE matmuls run in multi-pass LOW_HIGH
mode and are not competitive.

Raw (non-Tile) implementation: hand-placed semaphores keep the setup and
teardown overhead minimal (Tile's vector-clock epilogue resets every
semaphore individually, ~10us).
"""

import numpy as np
import ml_dtypes

import concourse.bass as bass
from concourse import bacc, mybir
from concourse.bass_utils import run_bass_kernel_spmd

B, C, S = 4, 2, 4096
K, NK, OUT = 64, 32, 512
Q = K * NK            # 2048 == S // 2
R = K + 1             # 65 shift taps
NCORES = 8
OSH = OUT // NCORES   # 64 output channels per core
KT = 32               # contraction tiles of 128 over c*Q = 4096
W260 = B * R          # per-kt rhs free dim: [z=0 | z=1] x [pair 0 | pair 1] x r
NCH = 4               # DMA chunks over kt
KTC = KT // NCH
CHUNK_W = KTC * (2 * OSH + W260)    # bf16 stream: [ch | cl | xh]
CHUNK8_W = KTC * (OSH + W260)       # fp8 stream:  [c8 | xl9]
CSC, XSC = 16.0, 512.0              # fp8 encode scales; product scale 2^13
F32 = mybir.dt.float32
BF16 = mybir.dt.bfloat16
F8 = mybir.dt.float8e4
GROUPS = [(0, 0), (0, 1), (1, 0), (1, 1)]  # (pr, p)

_PROGRAM = None
_LAST_RESULTS = None


def _build_program():
    nc = bacc.Bacc("TRN2", target_bir_lowering=False, debug=False)
    wbuf = nc.dram_tensor("wbuf", [128, NCH * CHUNK_W], BF16, kind="ExternalInput")
    # fp8 payload travels as uint8 (PJRT lacks float8_e4m3 support) and is
    # bitcast to fp8 at the matmul APs
    wbuf8 = nc.dram_tensor(
        "wbuf8", [128, NCH * CHUNK8_W], mybir.dt.uint8, kind="ExternalInput"
    )
    trow = nc.dram_tensor("trow", [1, Q], F32, kind="ExternalInput")
    bias2 = nc.dram_tensor("bias2", [128, 1], F32, kind="ExternalInput")
    ident = nc.dram_tensor("ident", [OSH, OSH], F32, kind="ExternalInput")
    out = nc.dram_tensor("out", [2 * 128, S], F32, kind="ExternalOutput")

    from contextlib import ExitStack

    with ExitStack() as ctx:
        e = ctx.enter_context
        wt = [e(nc.sbuf_tensor(f"wt{c}", [128, CHUNK_W], BF16)) for c in range(NCH)]
        wt8 = [
            e(nc.sbuf_tensor(f"wt8{c}", [128, CHUNK8_W], mybir.dt.uint8))
            for c in range(NCH)
        ]
        T_sb = e(nc.sbuf_tensor("T_sb", [128, Q], F32))
        b_sb = e(nc.sbuf_tensor("b_sb", [128, 1], F32))
        h_sb = e(nc.sbuf_tensor("h_sb", [OSH, W260], F32))
        id_sb = e(nc.sbuf_tensor("id_sb", [OSH, OSH], F32))
        tmp = [e(nc.sbuf_tensor(f"tmp{g}", [128, Q], F32)) for g in range(4)]
        o_sb = [e(nc.sbuf_tensor(f"o{g}", [128, Q], F32)) for g in range(4)]
        # pad h_ps to a full 2KB PSUM bank so h2_ps lands in its own bank
        h_ps_full = e(nc.psum_tensor("h_ps", [OSH, 512], F32))
        h_ps = h_ps_full[:, :W260]
        corr_ps_full = e(nc.psum_tensor("corr_ps", [OSH, 512], F32))
        corr_ps = corr_ps_full[:, :W260]
        h2_ps_full = e(nc.psum_tensor("h2_ps", [128, 512], F32))
        h2_ps = h2_ps_full[:, : 2 * R]

        sin = [e(nc.semaphore(f"sin{c}")) for c in range(NCH)]  # bf16 chunk DMAs
        sin8 = [e(nc.semaphore(f"si8{c}")) for c in range(NCH)]  # fp8 chunk DMAs
        sTa = e(nc.semaphore("sTa"))    # T broadcast DMA
        sTb = e(nc.semaphore("sTb"))    # bias DMA
        sid = e(nc.semaphore("sid"))    # identity DMA
        spe = e(nc.semaphore("spe"))    # mm1 done
        scp = e(nc.semaphore("scp"))    # h combine done
        scpa = e(nc.semaphore("scpa"))  # h psum->sbuf copy done
        sh2 = e(nc.semaphore("sh2"))    # h2 redistribution DMAs
        stt = e(nc.semaphore("stt"))    # DVE multiplies done (groups 0-2)
        sact = e(nc.semaphore("sact"))  # ACT relu done (groups 0-2)
        sdv = e(nc.semaphore("sdv"))    # DVE relu done (group 3)
        sout = e(nc.semaphore("sout"))  # out DMAs
        sems = sin + sin8 + [sTa, sTb, sid, spe, scp, scpa, sh2, stt, sact, sdv, sout]

        def exp_aps_half(g, hf):
            pr, p = GROUPS[g]
            off = pr * R + p + hf * (K // 2)
            h_exp = (
                h2_ps[:, off : off + K // 2]
                .unsqueeze(2)
                .to_broadcast((128, K // 2, NK))
            )
            sl = bass.ts(hf, Q // 2)
            return (
                tmp[g][:, sl].rearrange("a (m l) -> a m l", l=NK),
                h_exp,
                T_sb[:, sl].rearrange("a (m l) -> a m l", l=NK),
            )

        with nc.Block() as block:

            @block.sync
            def _(sync):
                for c in range(NCH):
                    sync.dma_start(
                        wt[c][:, :], wbuf[:, c * CHUNK_W : (c + 1) * CHUNK_W]
                    ).then_inc(sin[c], 16)
                    sync.dma_start(
                        wt8[c][:, :], wbuf8[:, c * CHUNK8_W : (c + 1) * CHUNK8_W]
                    ).then_inc(sin8[c], 16)
                # out DMAs: half-width units for earlier streaming
                n_act = 0
                n_dv = 0
                for g in range(4):
                    pr, p = GROUPS[g]
                    for hf in range(2):
                        if g < 3:
                            n_act += 1
                            sync.wait_ge(sact, n_act)
                        else:
                            n_dv += 1
                            sync.wait_ge(sdv, n_dv)
                        sync.dma_start(
                            out[
                                pr * 128 : (pr + 1) * 128,
                                p * Q + hf * (Q // 2) : p * Q + (hf + 1) * (Q // 2),
                            ],
                            o_sb[g][:, bass.ts(hf, Q // 2)],
                        ).then_inc(sout, 16)

            @block.scalar
            def _(scalar):
                scalar.dma_start(b_sb[:, :], bias2[:, :]).then_inc(sTb, 16)
                scalar.dma_start(id_sb[:, :], ident[:, :]).then_inc(sid, 16)
                scalar.dma_start(
                    T_sb[:, :], trow[:, :].to_broadcast((128, Q))
                ).then_inc(sTa, 16)
                scalar.wait_ge(sTb, 16)
                n_tt = 0
                n_act = 0
                for g in range(3):
                    for hf in range(2):
                        n_tt += 1
                        scalar.wait_ge(stt, n_tt)
                        n_act += 1
                        scalar.activation(
                            o_sb[g][:, bass.ts(hf, Q // 2)],
                            tmp[g][:, bass.ts(hf, Q // 2)],
                            mybir.ActivationFunctionType.Relu, bias=b_sb[:, :],
                        ).then_inc(sact, 1)

            @block.tensor
            def _(tensor):
                xo = 2 * KTC * OSH          # xh offset in bf16 chunk
                x8o = KTC * OSH             # xl9 offset in fp8 chunk
                n_main = NCH * KTC * 2
                n_corr = NCH * KTC
                i_main = i_corr = 0
                for chk in range(NCH):
                    tensor.wait_ge(sin[chk], 16)
                    tensor.wait_ge(sin8[chk], 16)
                    for kt in range(KTC):
                        xh_t = wt[chk][:, xo + kt * W260 : xo + (kt + 1) * W260]
                        for c_off in (0, KTC * OSH):
                            tensor.matmul(
                                h_ps[:, :],
                                wt[chk][
                                    :, c_off + kt * OSH : c_off + (kt + 1) * OSH
                                ],
                                xh_t,
                                start=(i_main == 0),
                                stop=(i_main == n_main - 1),
                            )
                            i_main += 1
                        mm = tensor.matmul(
                            corr_ps[:, :],
                            wt8[chk][:, kt * OSH : (kt + 1) * OSH].bitcast(F8),
                            wt8[chk][
                                :, x8o + kt * W260 : x8o + (kt + 1) * W260
                            ].bitcast(F8),
                            start=(i_corr == 0),
                            stop=(i_corr == n_corr - 1),
                        )
                        if i_corr == n_corr - 1:
                            mm.then_inc(spe, 1)
                        i_corr += 1
                # redistribute h (64, [z|pr|r]) -> h2 (z*64+o', pr*65+r) with
                # identity matmuls on the (already warm) PE: no DMA receipt.
                tensor.wait_ge(sid, 16)
                for z in range(2):
                    tensor.wait_ge(scp, z + 1)
                    tensor.matmul(
                        h2_ps[z * OSH : (z + 1) * OSH, :],
                        id_sb[:, :],
                        h_sb[:, z * 2 * R : (z + 1) * 2 * R],
                        start=True,
                        stop=True,
                    ).then_inc(sh2, 1)

            @block.vector
            def _(vector):
                vector.wait_ge(spe, 1)
                for z in range(2):
                    sl = slice(z * 2 * R, (z + 1) * 2 * R)
                    vector.tensor_copy(h_sb[:, sl], h_ps[:, sl]).then_inc(scpa, 1)
                for z in range(2):
                    sl = slice(z * 2 * R, (z + 1) * 2 * R)
                    vector.wait_ge(scpa, z + 1)
                    vector.scalar_tensor_tensor(
                        h_sb[:, sl], corr_ps[:, sl], 2.0 ** -13, h_sb[:, sl],
                        mybir.AluOpType.mult, mybir.AluOpType.add,
                    ).then_inc(scp, 1)
                vector.wait_ge(sh2, 2)
                vector.wait_ge(sTa, 16)
                n_tt = 0
                for g in range(4):
                    for hf in range(2):
                        o, i0, i1 = exp_aps_half(g, hf)
                        tt = vector.tensor_tensor(
                            o, i0, i1, mybir.AluOpType.mult
                        ).then_inc(stt, 1)
                        n_tt += 1
                        if g == 3:
                            if hf == 0:
                                vector.wait_ge(sTb, 16)
                            vector.wait_ge(stt, n_tt)
                            vector.tensor_scalar(
                                o_sb[3][:, bass.ts(hf, Q // 2)],
                                tmp[3][:, bass.ts(hf, Q // 2)],
                                b_sb[:, :], 0.0,
                                mybir.AluOpType.add, mybir.AluOpType.max,
                            ).then_inc(sdv, 1)

    nc.compile()
    return nc


def _split_bf16(a):
    hi = a.astype(ml_dtypes.bfloat16)
    lo = (a - hi.astype(np.float32)).astype(ml_dtypes.bfloat16)
    return hi, lo


def _host_prepare(x, stft_w, conv_w, conv_b):
    """Build per-core input maps."""
    x = np.ascontiguousarray(x, dtype=np.float32)
    xp = np.zeros((B, C, 2 * Q + K), dtype=np.float32)  # padded to 4160
    xp[:, :, :S] = x
    sb_, sc_, ss_ = xp.strides
    win = np.lib.stride_tricks.as_strided(
        xp, shape=(B, C, Q, R), strides=(sb_, sc_, 2 * ss_, ss_)
    )
    Xf = win.reshape(B, C * Q, R)                      # (4, 4096, 65), b=2*pr+z
    # layout [p, kt, z, pr, r]: batch order (z,pr) -> b = [0, 2, 1, 3]
    X5 = np.ascontiguousarray(
        Xf[[0, 2, 1, 3]].reshape(2, 2, KT, 128, R).transpose(3, 2, 0, 1, 4)
    ).reshape(128, KT, W260)
    xh, xl = _split_bf16(X5)
    xl9 = np.ascontiguousarray(
        (xl.astype(np.float32) * XSC)
    ).astype(ml_dtypes.float8_e4m3)

    trow = np.ascontiguousarray(stft_w.T, dtype=np.float32).reshape(1, Q)

    in_maps = []
    for i in range(NCORES):
        cw_sh = conv_w[i * OSH : (i + 1) * OSH, :]     # (64, 4096)
        cwt = np.ascontiguousarray(
            cw_sh.reshape(OSH, KT, 128).transpose(2, 1, 0)  # (128, 32, 64)
        )
        ch, cl = _split_bf16(cwt)
        c8 = (ch.astype(np.float32) * CSC).astype(ml_dtypes.float8_e4m3)
        wbuf = np.empty((128, NCH, CHUNK_W), dtype=ml_dtypes.bfloat16)
        wbuf8 = np.empty((128, NCH, CHUNK8_W), dtype=ml_dtypes.float8_e4m3)
        for chk in range(NCH):
            sl = slice(chk * KTC, (chk + 1) * KTC)
            wbuf[:, chk, :] = np.concatenate(
                [p_[:, sl].reshape(128, -1) for p_ in (ch, cl, xh)], axis=1
            )
            wbuf8[:, chk, :] = np.concatenate(
                [p_[:, sl].reshape(128, -1) for p_ in (c8, xl9)], axis=1
            )
        bias2 = np.ascontiguousarray(
            np.tile(conv_b[i * OSH : (i + 1) * OSH], 2).reshape(128, 1),
            dtype=np.float32,
        )
        in_maps.append(
            {
                "wbuf": wbuf.reshape(128, NCH * CHUNK_W),
                "wbuf8": wbuf8.reshape(128, NCH * CHUNK8_W).view(np.uint8),
                "trow": trow,
                "bias2": bias2,
                "ident": np.eye(OSH, dtype=np.float32),
            }
        )
    return in_maps


def kernel(x, stft_w, conv_w, conv_b):
    global _PROGRAM, _LAST_RESULTS
    if _PROGRAM is None:
        _PROGRAM = _build_program()
    in_maps = _host_prepare(
        np.asarray(x), np.asarray(stft_w, dtype=np.float32),
        np.asarray(conv_w, dtype=np.float32), np.asarray(conv_b, dtype=np.float32),
    )
    res = run_bass_kernel_spmd(_PROGRAM, in_maps, list(range(NCORES)))
    _LAST_RESULTS = res
    # per-core out: (256, 4096); rows pr*128 + z*64 + o' -> (b=2*pr+z, o=i*64+o')
    full = np.empty((B, OUT, S), dtype=np.float32)
    for i in range(NCORES):
        full[:, i * OSH : (i + 1) * OSH, :] = res.results[i]["out"].reshape(
            B, OSH, S
        )
    return full


if __name__ == "__main__":
    rng = np.random.default_rng(0)
    out = kernel(
        rng.standard_normal((B, C, S), dtype=np.float32),
        rng.standard_normal((NK, K), dtype=np.float32),
        (rng.standard_normal((OUT, C * K * NK)) * 0.02).astype(np.float32),
        (rng.standard_normal((OUT,)) * 0.02).astype(np.float32),
    )
    print(out.shape, out.dtype, float(np.abs(out).max()))


# revision 21
# speedup vs baseline: 1.3191x; 1.0403x over previous
"""Trainium2 Bass kernel for nn_CustomCNNLayer_84559316124470.

The reference computes, per batch b:
    win[b,c,s,m]   = xp[b,c,s+m]                    (xp = x padded with K-1 zeros)
    xw[b,c,s,m,l]  = win[b,c,s,m] * stft_w[l,m]
    xr             = xw.reshape(b, c*K*NK, s)       (raw row-major reshape)
    out            = relu(conv_w @ xr + bias)       (1x1 conv over channels)

Because K*NK == S/2 == 2048, the raw reshape maps
    xr[b, c*2048 + q, p*2048 + m*32 + l] = xp[b, c, 2q+p+m] * stft_w[l, m]
(with s = 2q+p). Hence, with h[b,o,r] = sum_{c,q} conv_w[o, c*2048+q] * xp[b,c,2q+r]
(r in [0, 65)):
    out[b, o, p*2048 + m*32 + l] = relu(stft_w[l,m] * h[b,o,p+m] + bias[o])

So the dense 8.6 GMAC/batch matmul collapses to a (512x4096)@(4096x65)
strided correlation (tensor engine) plus a per-element broadcast expansion
(vector/gpsimd engines) and bias+ReLU (scalar/vector engines).

Sharding: output channels o split across the 8 cores (64 rows each);
window matrices replicated. No collectives.

Precision: mm1 runs on the PE in bf16. With PASSES=3 the fp32 operands are
split hi/lo into bf16 pairs and three accumulating matmuls recover ~fp32
accuracy (error ~1e-5 rel.); fp32 PE matmuls run in multi-pass LOW_HIGH
mode and are not competitive.

Raw (non-Tile) implementation: hand-placed semaphores keep the setup and
teardown overhead minimal (Tile's vector-clock epilogue resets every
semaphore individually, ~10us).
"""

import numpy as np
import ml_dtypes

import concourse.bass as bass
from concourse import bacc, mybir
from concourse.bass_utils import run_bass_kernel_spmd

B, C, S = 4, 2, 4096
K, NK, OUT = 64, 32, 512
Q = K * NK            # 2048 == S // 2
R = K + 1             # 65 shift taps
NCORES = 8
OSH = OUT // NCORES   # 64 output channels per core
KT = 32               # contraction tiles of 128 over c*Q = 4096
W260 = B * R          # per-kt rhs free dim: [z=0 | z=1] x [pair 0 | pair 1] x r
NCH = 8               # DMA chunks over kt
KTC = KT // NCH
CHUNK_W = KTC * (2 * OSH + W260)    # bf16 stream: [ch | cl | xh]
CHUNK8_W = KTC * (OSH + W260)       # fp8 stream:  [c8 | xl9]
CSC, XSC = 16.0, 512.0              # fp8 encode scales; product scale 2^13
F32 = mybir.dt.float32
BF16 = mybir.dt.bfloat16
F8 = mybir.dt.float8e4
GROUPS = [(0, 0), (0, 1), (1, 0), (1, 1)]  # (pr, p)

_PROGRAM = None
_LAST_RESULTS = None


def _build_program():
    nc = bacc.Bacc("TRN2", target_bir_lowering=False, debug=False)
    wbuf = nc.dram_tensor("wbuf", [128, NCH * CHUNK_W], BF16, kind="ExternalInput")
    # fp8 payload travels as uint8 (PJRT lacks float8_e4m3 support) and is
    # bitcast to fp8 at the matmul APs
    wbuf8 = nc.dram_tensor(
        "wbuf8", [128, NCH * CHUNK8_W], mybir.dt.uint8, kind="ExternalInput"
    )
    trow = nc.dram_tensor("trow", [1, Q], F32, kind="ExternalInput")
    bias2 = nc.dram_tensor("bias2", [128, 1], F32, kind="ExternalInput")
    ident = nc.dram_tensor("ident", [OSH, OSH], F32, kind="ExternalInput")
    out = nc.dram_tensor("out", [2 * 128, S], F32, kind="ExternalOutput")

    from contextlib import ExitStack

    with ExitStack() as ctx:
        e = ctx.enter_context
        wt = [e(nc.sbuf_tensor(f"wt{c}", [128, CHUNK_W], BF16)) for c in range(NCH)]
        wt8 = [
            e(nc.sbuf_tensor(f"wt8{c}", [128, CHUNK8_W], mybir.dt.uint8))
            for c in range(NCH)
        ]
        T_sb = e(nc.sbuf_tensor("T_sb", [128, Q], F32))
        b_sb = e(nc.sbuf_tensor("b_sb", [128, 1], F32))
        h_sb = e(nc.sbuf_tensor("h_sb", [OSH, W260], F32))
        id_sb = e(nc.sbuf_tensor("id_sb", [OSH, OSH], F32))
        tmp = [e(nc.sbuf_tensor(f"tmp{g}", [128, Q], F32)) for g in range(4)]
        o_sb = [e(nc.sbuf_tensor(f"o{g}", [128, Q], F32)) for g in range(4)]
        # pad h_ps to a full 2KB PSUM bank so h2_ps lands in its own bank
        h_ps_full = e(nc.psum_tensor("h_ps", [OSH, 512], F32))
        h_ps = h_ps_full[:, :W260]
        corr_ps_full = e(nc.psum_tensor("corr_ps", [OSH, 512], F32))
        corr_ps = corr_ps_full[:, :W260]
        h2_ps_full = e(nc.psum_tensor("h2_ps", [128, 512], F32))
        h2_ps = h2_ps_full[:, : 2 * R]

        sin = [e(nc.semaphore(f"sin{c}")) for c in range(NCH)]  # bf16 chunk DMAs
        sin8 = [e(nc.semaphore(f"si8{c}")) for c in range(NCH)]  # fp8 chunk DMAs
        sTa = e(nc.semaphore("sTa"))    # T broadcast DMA
        sTb = e(nc.semaphore("sTb"))    # bias DMA
        sid = e(nc.semaphore("sid"))    # identity DMA
        spe = e(nc.semaphore("spe"))    # mm1 done
        scp = e(nc.semaphore("scp"))    # h combine done
        scpa = e(nc.semaphore("scpa"))  # h psum->sbuf copy done
        sh2 = e(nc.semaphore("sh2"))    # h2 redistribution DMAs
        stt = e(nc.semaphore("stt"))    # DVE multiplies done (groups 0-2)
        sact = e(nc.semaphore("sact"))  # ACT relu done (groups 0-2)
        sdv = e(nc.semaphore("sdv"))    # DVE relu done (group 3)
        sout = e(nc.semaphore("sout"))  # out DMAs
        sems = sin + sin8 + [sTa, sTb, sid, spe, scp, scpa, sh2, stt, sact, sdv, sout]

        def exp_aps_half(g, hf):
            pr, p = GROUPS[g]
            off = pr * R + p + hf * (K // 2)
            h_exp = (
                h2_ps[:, off : off + K // 2]
                .unsqueeze(2)
                .to_broadcast((128, K // 2, NK))
            )
            sl = bass.ts(hf, Q // 2)
            return (
                tmp[g][:, sl].rearrange("a (m l) -> a m l", l=NK),
                h_exp,
                T_sb[:, sl].rearrange("a (m l) -> a m l", l=NK),
            )

        with nc.Block() as block:

            @block.sync
            def _(sync):
                for c in range(NCH):
                    sync.dma_start(
                        wt[c][:, :], wbuf[:, c * CHUNK_W : (c + 1) * CHUNK_W]
                    ).then_inc(sin[c], 16)
                    sync.dma_start(
                        wt8[c][:, :], wbuf8[:, c * CHUNK8_W : (c + 1) * CHUNK8_W]
                    ).then_inc(sin8[c], 16)
                # out DMAs: half-width units for earlier streaming
                n_act = 0
                n_dv = 0
                for g in range(4):
                    pr, p = GROUPS[g]
                    for hf in range(2):
                        if g < 3:
                            n_act += 1
                            sync.wait_ge(sact, n_act)
                        else:
                            n_dv += 1
                            sync.wait_ge(sdv, n_dv)
                        sync.dma_start(
                            out[
                                pr * 128 : (pr + 1) * 128,
                                p * Q + hf * (Q // 2) : p * Q + (hf + 1) * (Q // 2),
                            ],
                            o_sb[g][:, bass.ts(hf, Q // 2)],
                        ).then_inc(sout, 16)

            @block.scalar
            def _(scalar):
                scalar.dma_start(b_sb[:, :], bias2[:, :]).then_inc(sTb, 16)
                scalar.dma_start(id_sb[:, :], ident[:, :]).then_inc(sid, 16)
                # delay the 1MB T broadcast until chunk0 lands so it does
                # not compete with the mm1-critical input stream
                scalar.wait_ge(sin8[0], 16)
                scalar.dma_start(
                    T_sb[:, :], trow[:, :].to_broadcast((128, Q))
                ).then_inc(sTa, 16)
                scalar.wait_ge(sTb, 16)
                n_tt = 0
                n_act = 0
                for g in range(3):
                    for hf in range(2):
                        n_tt += 1
                        scalar.wait_ge(stt, n_tt)
                        n_act += 1
                        scalar.activation(
                            o_sb[g][:, bass.ts(hf, Q // 2)],
                            tmp[g][:, bass.ts(hf, Q // 2)],
                            mybir.ActivationFunctionType.Relu, bias=b_sb[:, :],
                        ).then_inc(sact, 1)

            @block.tensor
            def _(tensor):
                xo = 2 * KTC * OSH          # xh offset in bf16 chunk
                x8o = KTC * OSH             # xl9 offset in fp8 chunk
                n_main = NCH * KTC * 2
                n_corr = NCH * KTC
                i_main = i_corr = 0
                for chk in range(NCH):
                    tensor.wait_ge(sin[chk], 16)
                    tensor.wait_ge(sin8[chk], 16)
                    for kt in range(KTC):
                        xh_t = wt[chk][:, xo + kt * W260 : xo + (kt + 1) * W260]
                        for c_off in (0, KTC * OSH):
                            tensor.matmul(
                                h_ps[:, :],
                                wt[chk][
                                    :, c_off + kt * OSH : c_off + (kt + 1) * OSH
                                ],
                                xh_t,
                                start=(i_main == 0),
                                stop=(i_main == n_main - 1),
                            )
                            i_main += 1
                        mm = tensor.matmul(
                            corr_ps[:, :],
                            wt8[chk][:, kt * OSH : (kt + 1) * OSH].bitcast(F8),
                            wt8[chk][
                                :, x8o + kt * W260 : x8o + (kt + 1) * W260
                            ].bitcast(F8),
                            start=(i_corr == 0),
                            stop=(i_corr == n_corr - 1),
                        )
                        if i_corr == n_corr - 1:
                            mm.then_inc(spe, 1)
                        i_corr += 1
                # redistribute h (64, [z|pr|r]) -> h2 (z*64+o', pr*65+r) with
                # identity matmuls on the (already warm) PE: no DMA receipt.
                tensor.wait_ge(sid, 16)
                for z in range(2):
                    tensor.wait_ge(scp, z + 1)
                    tensor.matmul(
                        h2_ps[z * OSH : (z + 1) * OSH, :],
                        id_sb[:, :],
                        h_sb[:, z * 2 * R : (z + 1) * 2 * R],
                        start=True,
                        stop=True,
                    ).then_inc(sh2, 1)

            @block.vector
            def _(vector):
                vector.wait_ge(spe, 1)
                for z in range(2):
                    sl = slice(z * 2 * R, (z + 1) * 2 * R)
                    vector.tensor_copy(h_sb[:, sl], h_ps[:, sl]).then_inc(scpa, 1)
                for z in range(2):
                    sl = slice(z * 2 * R, (z + 1) * 2 * R)
                    vector.wait_ge(scpa, z + 1)
                    vector.scalar_tensor_tensor(
                        h_sb[:, sl], corr_ps[:, sl], 2.0 ** -13, h_sb[:, sl],
                        mybir.AluOpType.mult, mybir.AluOpType.add,
                    ).then_inc(scp, 1)
                vector.wait_ge(sh2, 2)
                vector.wait_ge(sTa, 16)
                n_tt = 0
                for g in range(4):
                    for hf in range(2):
                        o, i0, i1 = exp_aps_half(g, hf)
                        tt = vector.tensor_tensor(
                            o, i0, i1, mybir.AluOpType.mult
                        ).then_inc(stt, 1)
                        n_tt += 1
                        if g == 3:
                            if hf == 0:
                                vector.wait_ge(sTb, 16)
                            vector.wait_ge(stt, n_tt)
                            vector.tensor_scalar(
                                o_sb[3][:, bass.ts(hf, Q // 2)],
                                tmp[3][:, bass.ts(hf, Q // 2)],
                                b_sb[:, :], 0.0,
                                mybir.AluOpType.add, mybir.AluOpType.max,
                            ).then_inc(sdv, 1)

    nc.compile()
    return nc


def _split_bf16(a):
    hi = a.astype(ml_dtypes.bfloat16)
    lo = (a - hi.astype(np.float32)).astype(ml_dtypes.bfloat16)
    return hi, lo


def _host_prepare(x, stft_w, conv_w, conv_b):
    """Build per-core input maps."""
    x = np.ascontiguousarray(x, dtype=np.float32)
    xp = np.zeros((B, C, 2 * Q + K), dtype=np.float32)  # padded to 4160
    xp[:, :, :S] = x
    sb_, sc_, ss_ = xp.strides
    win = np.lib.stride_tricks.as_strided(
        xp, shape=(B, C, Q, R), strides=(sb_, sc_, 2 * ss_, ss_)
    )
    Xf = win.reshape(B, C * Q, R)                      # (4, 4096, 65), b=2*pr+z
    # layout [p, kt, z, pr, r]: batch order (z,pr) -> b = [0, 2, 1, 3]
    X5 = np.ascontiguousarray(
        Xf[[0, 2, 1, 3]].reshape(2, 2, KT, 128, R).transpose(3, 2, 0, 1, 4)
    ).reshape(128, KT, W260)
    xh, xl = _split_bf16(X5)
    xl9 = np.ascontiguousarray(
        (xl.astype(np.float32) * XSC)
    ).astype(ml_dtypes.float8_e4m3)

    trow = np.ascontiguousarray(stft_w.T, dtype=np.float32).reshape(1, Q)

    in_maps = []
    for i in range(NCORES):
        cw_sh = conv_w[i * OSH : (i + 1) * OSH, :]     # (64, 4096)
        cwt = np.ascontiguousarray(
            cw_sh.reshape(OSH, KT, 128).transpose(2, 1, 0)  # (128, 32, 64)
        )
        ch, cl = _split_bf16(cwt)
        c8 = (ch.astype(np.float32) * CSC).astype(ml_dtypes.float8_e4m3)
        wbuf = np.empty((128, NCH, CHUNK_W), dtype=ml_dtypes.bfloat16)
        wbuf8 = np.empty((128, NCH, CHUNK8_W), dtype=ml_dtypes.float8_e4m3)
        for chk in range(NCH):
            sl = slice(chk * KTC, (chk + 1) * KTC)
            wbuf[:, chk, :] = np.concatenate(
                [p_[:, sl].reshape(128, -1) for p_ in (ch, cl, xh)], axis=1
            )
            wbuf8[:, chk, :] = np.concatenate(
                [p_[:, sl].reshape(128, -1) for p_ in (c8, xl9)], axis=1
            )
        bias2 = np.ascontiguousarray(
            np.tile(conv_b[i * OSH : (i + 1) * OSH], 2).reshape(128, 1),
            dtype=np.float32,
        )
        in_maps.append(
            {
                "wbuf": wbuf.reshape(128, NCH * CHUNK_W),
                "wbuf8": wbuf8.reshape(128, NCH * CHUNK8_W).view(np.uint8),
                "trow": trow,
                "bias2": bias2,
                "ident": np.eye(OSH, dtype=np.float32),
            }
        )
    return in_maps


def kernel(x, stft_w, conv_w, conv_b):
    global _PROGRAM, _LAST_RESULTS
    if _PROGRAM is None:
        _PROGRAM = _build_program()
    in_maps = _host_prepare(
        np.asarray(x), np.asarray(stft_w, dtype=np.float32),
        np.asarray(conv_w, dtype=np.float32), np.asarray(conv_b, dtype=np.float32),
    )
    res = run_bass_kernel_spmd(_PROGRAM, in_maps, list(range(NCORES)))
    _LAST_RESULTS = res
    # per-core out: (256, 4096); rows pr*128 + z*64 + o' -> (b=2*pr+z, o=i*64+o')
    full = np.empty((B, OUT, S), dtype=np.float32)
    for i in range(NCORES):
        full[:, i * OSH : (i + 1) * OSH, :] = res.results[i]["out"].reshape(
            B, OSH, S
        )
    return full


if __name__ == "__main__":
    rng = np.random.default_rng(0)
    out = kernel(
        rng.standard_normal((B, C, S), dtype=np.float32),
        rng.standard_normal((NK, K), dtype=np.float32),
        (rng.standard_normal((OUT, C * K * NK)) * 0.02).astype(np.float32),
        (rng.standard_normal((OUT,)) * 0.02).astype(np.float32),
    )
    print(out.shape, out.dtype, float(np.abs(out).max()))
